# revision 1
# baseline (speedup 1.0000x reference)
"""Trainium2 Bass kernel for nn_DecoderVectorized (axial decoder with causal
cross-attention). Self-contained: hardcodes all shapes/sharding.

Sharding: 32 SPMD slots = 8 cores x 4 slots over the B*(T-1)=30 frames
(sorted by t so slot j has kv prefix 128*{4,8,12,15}; 2 dummy slots).
"""
import math
import sys

import numpy as np

try:
    import concourse.bass as bass
except ImportError:  # pragma: no cover
    sys.path.insert(0, "/opt/trn_rl_repo")
    import concourse.bass as bass

import concourse.bacc as bacc
import concourse.mybir as mybir
import concourse.tile as tile
from concourse import bass_utils
from concourse.masks import make_identity

F32 = mybir.dt.float32
F32R = mybir.dt.float32r
BF16 = mybir.dt.bfloat16
import ml_dtypes
NPBF = ml_dtypes.bfloat16
AF = mybir.ActivationFunctionType
OP = mybir.AluOpType

H, W, D, HEADS, QL = 16, 16, 192, 8, 256
B, T, M = 2, 16, 128
NQ = H * W          # 256 tokens per frame
DH = D // HEADS     # 24
NCORE, NSLOT = 8, 4
TMAX = [4, 8, 12, 16]
KV = [t * 128 for t in TMAX]        # 512 1024 1536 2048
CM = 32.0                           # mask bias (power of 2: bf16-exact)
SCL = 1.0 / math.sqrt(DH)
EPS = 1e-5


def _frame(f):
    """frame index f in [0,32) -> (b, t); 30/31 are dummies."""
    if f >= 30:
        return (f - 30, 15)
    return (f % 2, f // 2 + 1)


# ---------------------------------------------------------------- host prep

def _qk_colmat_s1(w, bvec, g, is_q):
    """[193,128] colmat for stage-1 qhT/khT half g. Head h data at cols
    32h+1..32h+24 (aux row 0 first); col 32h+0: e_192 (ones) if is_q."""
    m = np.zeros((193, 128), np.float32)
    for h in range(4):
        Hh = 4 * g + h
        m[0:192, 32 * h + 1:32 * h + 25] = w[:, DH * Hh:DH * Hh + DH]
        m[192, 32 * h + 1:32 * h + 25] = bvec[DH * Hh:DH * Hh + DH]
        if is_q:
            m[192, 32 * h] = 1.0
    return m


def _qk_colmat_23(w, bvec, ch, g):
    """[97,128] colmat chunk for stage-2/3 qhT/khT. Data at cols 32h+3..+26;
    aux cols 0..2 zero (DMA'd)."""
    m = np.zeros((97, 128), np.float32)
    for h in range(4):
        Hh = 4 * g + h
        m[0:96, 32 * h + 3:32 * h + 27] = w[96 * ch:96 * ch + 96, DH * Hh:DH * Hh + DH]
        m[96, 32 * h + 3:32 * h + 27] = bvec[DH * Hh:DH * Hh + DH] * 0.5
    return m


def _wv_colmat_s1(w, bvec):
    """[193,256]: head Hh data at cols 32Hh..+23, ones col at 32Hh+24."""
    m = np.zeros((193, 256), np.float32)
    for Hh in range(8):
        m[0:192, 32 * Hh:32 * Hh + 24] = w[:, DH * Hh:DH * Hh + DH]
        m[192, 32 * Hh:32 * Hh + 24] = bvec[DH * Hh:DH * Hh + DH]
        m[192, 32 * Hh + 24] = 1.0
    return m


def _wv_colmat_23(w, bvec, ch):
    m = np.zeros((97, 256), np.float32)
    for Hh in range(8):
        m[0:96, 32 * Hh:32 * Hh + 24] = w[96 * ch:96 * ch + 96, DH * Hh:DH * Hh + DH]
        m[96, 32 * Hh:32 * Hh + 24] = bvec[DH * Hh:DH * Hh + DH] * 0.5
        m[96, 32 * Hh + 24] = 0.5
    return m


def _wo_aug(w, bvec):
    """[128, 384]: head H=4g+h at partition rows 32h..32h+31, col block 192g:
    rows 0..23 = wo rows, row 24 = bo/8 (walrus needs lhsT/rhs same base)."""
    m = np.zeros((128, 2 * D), np.float32)
    for g in range(2):
        for h in range(4):
            Hh = 4 * g + h
            m[32 * h:32 * h + 24, D * g:D * g + D] = w[DH * Hh:DH * Hh + DH, :]
            m[32 * h + 24, D * g:D * g + D] = bvec / 8.0
    return m


def _aux_rows(idx, is_q):
    """[128,256] aux contraction rows for the rank-3 axial mask, pre-spread to
    partition rows 32h+0..2. k-side: [ri^2, ri, 1]; q-side: [-c, 2c rj, -c rj^2]."""
    r = idx.astype(np.float32)
    if is_q:
        rows = np.stack([np.full(NQ, -CM, np.float32), 2.0 * CM * r, -CM * r * r])
    else:
        rows = np.stack([r * r, r, np.ones(NQ, np.float32)])
    m = np.zeros((128, NQ), np.float32)
    for h in range(4):
        m[32 * h:32 * h + 3] = rows
    return m


def _host_constants(inp):
    """Shared (core-independent) device constant arrays."""
    c = {}
    g, b_ = inp["rn_g"], inp["rn_b"]

    def eff(wq, bq, scale):
        return (g[:, None] * wq * scale).astype(np.float32), \
               ((b_ @ wq + bq) * scale).astype(np.float32)

    for gg in range(2):
        c[f"cwq{gg}"] = _qk_colmat_s1(inp["c_wq"] * SCL, inp["c_bq"] * SCL, gg, True)
        c[f"cwk{gg}"] = _qk_colmat_s1(inp["c_wk"], inp["c_bk"], gg, False)
    c["cwv"] = _wv_colmat_s1(inp["c_wv"], inp["c_bv"])
    c["cwo"] = _wo_aug(inp["c_wo"], inp["c_bo"])
    tok = np.arange(NQ)
    for pre, wp, idx in (("r", "r", tok // 16), ("l", "col", tok % 16)):
        wq, bq = eff(inp[wp + "_wq"], inp[wp + "_bq"], SCL)
        wk, bk = eff(inp[wp + "_wk"], inp[wp + "_bk"], 1.0)
        wv, bv = eff(inp[wp + "_wv"], inp[wp + "_bv"], 1.0)
        for ch in range(2):
            for gg in range(2):
                c[f"{pre}wq{ch}{gg}"] = _qk_colmat_23(wq, bq, ch, gg)
                c[f"{pre}wk{ch}{gg}"] = _qk_colmat_23(wk, bk, ch, gg)
            c[f"{pre}wv{ch}"] = _wv_colmat_23(wv, bv, ch)
        c[f"{pre}wo"] = _wo_aug(inp[wp + "_wo"], inp[wp + "_bo"])
        c[f"{pre}ka"] = _aux_rows(idx, False)
        c[f"{pre}qa"] = _aux_rows(idx, True)
    w1 = (inp["ff_ln_g"][:, None] * inp["ff_w1"]).astype(np.float32)
    b1 = (inp["ff_ln_b"] @ inp["ff_w1"] + inp["ff_b1"]).astype(np.float32)
    fw1 = np.zeros((2 * 97, 4 * D), np.float32)
    for ch in range(2):
        fw1[97 * ch:97 * ch + 96] = w1[96 * ch:96 * ch + 96]
        fw1[97 * ch + 96] = b1 * 0.5
    c["fw1"] = fw1
    fw2 = np.zeros((128, 6 * D), np.float32)
    for q in range(6):
        fw2[:, D * q:D * q + D] = inp["ff_w2"][128 * q:128 * q + 128, :]
    c["fw2"] = fw2
    c["fb2"] = np.broadcast_to(inp["ff_b2"][None], (128, D)).copy().astype(np.float32)
    hw = np.zeros((96, 2 * QL), np.float32)
    hw[:, 0:QL] = inp["head_w"][0:96]
    hw[:, QL:2 * QL] = inp["head_w"][96:192]
    c["hw"] = hw
    c["hb"] = np.broadcast_to(inp["head_b"][None], (128, QL)).copy().astype(np.float32)
    sel = np.zeros((128, 4), np.float32)
    for h in range(4):
        sel[32 * h + 24, h] = 1.0
    c["sel"] = sel
    rp = np.zeros((32, 8 * 128), np.float32)
    for k in range(8):
        for h in range(4):
            rp[4 * k + h, 128 * k + 32 * h:128 * k + 32 * h + 32] = 1.0
    c["repl"] = rp
    return c


def _core_inputs(inp, const, core):
    """Per-core in_map (includes the shared consts)."""
    m = dict(const)
    qg = np.asarray(inp["query_grid"], np.float32)
    tp = np.asarray(inp["t_pos_w"], np.float32)
    mt = np.asarray(inp["mem_tokens"], np.float32)
    qT = np.zeros((NSLOT * 193, NQ), np.float32)
    for j in range(NSLOT):
        b, t = _frame(8 * j + core)
        qT[193 * j:193 * j + 192] = (qg + tp[t][None, :]).T
        qT[193 * j + 192] = 1.0
        kvT = np.ones((193, KV[j]), np.float32)
        kvT[0:192] = mt[b, :TMAX[j]].reshape(-1, D).T
        m[f"kvT{j}"] = kvT
        msk = np.zeros((4, KV[j]), np.float32)
        msk[:, 128 * t:] = -CM
        m[f"msk{j}"] = msk
    m["qT"] = qT
    return m


# ---------------------------------------------------------------- program

_CACHE = {}

# consts whose DRAM row-count exceeds 128: load as (rows0:97|0:96, rest) pairs
_SPLIT193 = ("cwq0", "cwq1", "cwk0", "cwk1", "cwv")


def build_program(gelu_f=AF.Gelu, debug=False, stop_stage=4):
    key = (gelu_f, debug, stop_stage)
    if key in _CACHE:
        return _CACHE[key]
    nc = bacc.Bacc("TRN2", target_bir_lowering=False, debug=False)

    # ---- DRAM I/O ----
    dr = {}
    def din(name, shape):
        dr[name] = nc.dram_tensor(name, shape, F32, kind="ExternalInput")
    for gg in range(2):
        din(f"cwq{gg}", (193, 128)); din(f"cwk{gg}", (193, 128))
    din("cwv", (193, 256)); din("cwo", (128, 2 * D))
    for pre in ("r", "l"):
        for ch in range(2):
            for gg in range(2):
                din(f"{pre}wq{ch}{gg}", (97, 128)); din(f"{pre}wk{ch}{gg}", (97, 128))
            din(f"{pre}wv{ch}", (97, 256))
        din(f"{pre}wo", (128, 2 * D))
        din(f"{pre}ka", (128, NQ)); din(f"{pre}qa", (128, NQ))
    din("fw1", (2 * 97, 4 * D)); din("fw2", (128, 6 * D)); din("fb2", (128, D))
    din("hw", (96, 2 * QL)); din("hb", (128, QL))
    din("sel", (128, 4)); din("repl", (32, 8 * 128))
    din("qT", (NSLOT * 193, NQ))
    for j in range(NSLOT):
        din(f"kvT{j}", (193, KV[j])); din(f"msk{j}", (4, KV[j]))
    out = nc.dram_tensor("out", (NSLOT * 2 * 128, QL), F32, kind="ExternalOutput")
    dbg = {}
    if debug:
        for nm, shape in (("d_qhT", (128, 256)), ("d_khT", (128, 512)),
                          ("d_pT", (128, 1024)), ("d_av", (128, 256)),
                          ("d_avn", (128, 256)), ("d_r1", (128, D)),
                          ("d_xh", (128, D)), ("d_xT", (97, 256)),
                          ("d_r2", (128, D)), ("d_r3", (128, D)),
                          ("d_h1g", (128, 256)), ("d_z", (128, D))):
            dbg[nm] = nc.dram_tensor(nm, shape, F32, kind="ExternalOutput")

    from contextlib import ExitStack
    with tile.TileContext(nc) as tc, ExitStack() as es:
        cst = es.enter_context(tc.tile_pool(name="cst", bufs=1))
        dyn = es.enter_context(tc.tile_pool(name="dyn", bufs=2))
        dy3 = es.enter_context(tc.tile_pool(name="dy3", bufs=3))
        dy8 = es.enter_context(tc.tile_pool(name="dy8", bufs=8))
        st = es.enter_context(tc.tile_pool(name="st", bufs=1))
        ps_s = es.enter_context(tc.tile_pool(name="ps_s", bufs=2, space="PSUM"))
        ps_a = es.enter_context(tc.tile_pool(name="ps_a", bufs=2, space="PSUM"))
        ps_g = es.enter_context(tc.tile_pool(name="ps_g", bufs=2, space="PSUM"))

        def rr(ap):
            return ap.bitcast(mybir.dt.float32r)

        def pg(p_, f_):
            return ps_g.tile([p_, f_], F32, tag="pg", name="pg")

        # ---- load constants ----
        C = {}
        for nm, t_ in dr.items():
            if nm == "qT" or nm.startswith(("kvT", "msk")):
                continue
            shape = list(t_.shape)
            dt_ = t_.dtype
            if nm in _SPLIT193:
                ta = cst.tile([96, shape[1]], dt_, tag=nm + "a")
                tb = cst.tile([97, shape[1]], dt_, tag=nm + "b")
                nc.sync.dma_start(ta[:], t_.ap()[0:96, :])
                nc.sync.dma_start(tb[:], t_.ap()[96:193, :])
                C[nm] = (ta, tb)
            elif nm == "fw1":
                ta = cst.tile([97, shape[1]], dt_, tag="fw1a")
                tb = cst.tile([97, shape[1]], dt_, tag="fw1b")
                nc.sync.dma_start(ta[:], t_.ap()[0:97, :])
                nc.sync.dma_start(tb[:], t_.ap()[97:194, :])
                C[nm] = (ta, tb)
            else:
                tl = cst.tile(shape, dt_, tag=nm)
                nc.sync.dma_start(tl[:], t_.ap()[:])
                C[nm] = tl
        ident = cst.tile([128, 128], F32, tag="ident")
        make_identity(nc, ident[:])
        epsc = cst.tile([128, 1], F32, tag="epsc")
        nc.gpsimd.memset(epsc[:], EPS)

        def evac(dst_ap, src_ap):
            nc.vector.tensor_copy(dst_ap, src_ap)

        # ============ generic attention core ============
        def attn_core(iid, qsrc, ksrc, vsrc, kauxdma, qauxdma, kvlen, kp,
                      lstack, pres):
            """qsrc/ksrc: per half, list of (lhsT_chunk, rhs_chunk) pairs.
            vsrc(i): chunk pairs for kv-tile i. kp = rows/head (25 or 27).
            Returns av_sb (unnormalized ovT + l rows) per half."""
            nkt = kvlen // 128
            qhT, khT = [], []
            for g in range(2):
                pq = pg(128, 256)
                for i, (cm, rhs) in enumerate(qsrc[g]):
                    nc.tensor.matmul(pq[:], cm, rhs, start=(i == 0),
                                     stop=(i == len(qsrc[g]) - 1))
                q_sb = dyn.tile([128, 256], F32, tag=f"qhT{g}")
                evac(q_sb[:], pq[:])
                if qauxdma is not None:
                    for h in range(4):
                        nc.gpsimd.tensor_copy(q_sb[32 * h:32 * h + 3, :],
                                         qauxdma[32 * h:32 * h + 3, :])
                qhT.append(q_sb)
                k_sb = dyn.tile([128, kvlen], F32, tag=f"khT{g}", bufs=1)
                for nchunk in range(0, kvlen, 512):
                    nw = min(512, kvlen - nchunk)
                    pk = pg(128, 512)
                    for i, (cm, rhs) in enumerate(ksrc[g]):
                        nc.tensor.matmul(pk[:, 0:nw], cm,
                                         rhs[:, nchunk:nchunk + nw],
                                         start=(i == 0),
                                         stop=(i == len(ksrc[g]) - 1))
                    evac(k_sb[:, nchunk:nchunk + nw], pk[:, 0:nw])
                if kauxdma[1] == 1:       # stage-1 dynamic mask row (DRAM)
                    for h in range(4):
                        nc.gpsimd.dma_start(k_sb[32 * h:32 * h + 1, :],
                                            kauxdma[0][h:h + 1, :])
                else:                      # static axial-mask rows (SBUF)
                    for h in range(4):
                        nc.gpsimd.tensor_copy(k_sb[32 * h:32 * h + 3, :],
                                         kauxdma[0][32 * h:32 * h + 3, :])
                khT.append(k_sb)
            av = [ps_a.tile([128, 256], F32, tag="p_av", name="p_av") for _ in range(2)]
            npair = nkt // 2
            for ip in range(npair):
                vhs = []
                for d in range(2):
                    pv = pg(128, 256)
                    vch = vsrc(2 * ip + d)
                    for ci, (cm, rhs) in enumerate(vch):
                        nc.tensor.matmul(pv[:], cm, rhs, start=(ci == 0),
                                         stop=(ci == len(vch) - 1))
                    vh = dy3.tile([128, 256], F32, tag="vh")
                    evac(vh[:], pv[:])
                    vhs.append(vh)
                for g in range(2):
                    # head h in its own PSUM bank (cols 512h) -- concurrent
                    # row-group matmuls to one bank collide fatally on HW
                    pssT = ps_s.tile([128, 2048], F32, tag="p_sT", bufs=1)
                    for d in range(2):
                        for h in range(4):
                            nc.tensor.matmul(
                                pssT[:, 512 * h + 256 * d:512 * h + 256 * d + 256],
                                khT[g][32 * h:32 * h + kp,
                                       128 * (2 * ip + d):128 * (2 * ip + d) + 128],
                                qhT[g][32 * h:32 * h + kp, :],
                                start=True, stop=True, tile_position=(32 * h, 0))
                    pT = dy3.tile([128, 2048], F32, tag="pT", bufs=2)
                    nc.scalar.activation(pT[:], pssT[:], AF.Exp)
                    if debug and iid == 0 and ip == 0 and g == 0 and pres == "s1":
                        nc.gpsimd.dma_start(dbg["d_pT"].ap()[:], pT[:, 0:1024])
                    for d in range(2):
                        for h in range(4):
                            nc.tensor.matmul(
                                av[g][32 * h:32 * h + 32, :],
                                vhs[d][:, 32 * (4 * g + h):32 * (4 * g + h) + 32],
                                pT[:, 512 * h + 256 * d:512 * h + 256 * d + 256],
                                start=(ip == 0 and d == 0),
                                stop=(ip == npair - 1 and d == 1),
                                tile_position=(0, 32 * h), skip_group_check=True)
            av_sb = []
            for g in range(2):
                a_sb = dy8.tile([128, 256], F32, tag="av_sb")
                evac(a_sb[:], av[g][:])
                pl = pg(4, 256)
                nc.tensor.matmul(pl[:], C["sel"][:, 0:4], a_sb[:],
                                 start=True, stop=True)
                ltmp = dy3.tile([4, 256], F32, tag="ltmp")
                evac(ltmp[:], pl[:])
                nc.gpsimd.dma_start(lstack[8 * iid + 4 * g:8 * iid + 4 * g + 4, :],
                                    ltmp[:])
                av_sb.append(a_sb)
            if debug and iid == 0 and pres == "s1":
                nc.gpsimd.dma_start(dbg["d_qhT"].ap()[:], qhT[0][:])
                nc.gpsimd.dma_start(dbg["d_khT"].ap()[:], khT[0][:, 0:512])
                nc.sync.dma_start(dbg["d_av"].ap()[:], av_sb[0][:])
            return av_sb

        def attn_finish(iid, av_sb, rstack, wo_t, res_in, res_tag):
            """r-broadcast, normalize, wo projection (+residual)."""
            avn = []
            for g in range(2):
                prb = pg(128, 256)
                k = 2 * iid + g
                nc.tensor.matmul(prb[:], C["repl"][:, 128 * k:128 * k + 128],
                                 rstack[:], start=True, stop=True)
                an = dy3.tile([128, 256], F32, tag="avn")
                nc.vector.tensor_tensor(an[:], av_sb[g][:], prb[:], OP.mult)
                avn.append(an)
            outs = []
            for tt in range(2):
                py = pg(128, D)
                for g in range(2):
                    nc.tensor.matmul(
                        py[:], avn[g][:, 128 * tt:128 * tt + 128],
                        wo_t[:, D * g:D * g + D],
                        start=(g == 0), stop=(g == 1))
                r_new = st.tile([128, D], F32, tag=f"{res_tag}_{iid}_{tt}")
                if res_in is None:
                    evac(r_new[:], py[:])
                else:
                    nc.vector.tensor_tensor(r_new[:], res_in[tt][:], py[:], OP.add)
                outs.append(r_new)
            return outs

        # ============ LN helpers ============
        def ln_stats(x_tiles, name):
            # tensor_tensor_reduce is fatal at runtime on this stack; use
            # bn_stats/bn_aggr (one DVE pass -> mean, var per partition)
            n = len(x_tiles)
            var = dyn.tile([128, n], F32, tag=f"var_{name}")
            rs = dyn.tile([128, n], F32, tag=f"rs_{name}")
            aggs = []
            for k, xt in enumerate(x_tiles):
                bst = dy3.tile([128, 6], F32, tag="bst")
                nc.vector.bn_stats(bst[:], xt[:])
                agg = dy8.tile([128, 2], F32, tag=f"agg_{name}", name="agg")
                nc.vector.bn_aggr(agg[:], bst[:])
                nc.vector.tensor_copy(var[:, k:k + 1], agg[:, 1:2])
                aggs.append(agg)
            lnv = dyn.tile([128, n], F32, tag=f"lnv_{name}")
            nc.scalar.activation(lnv[:], var[:], AF.Ln, bias=epsc[:])
            nc.scalar.activation(rs[:], lnv[:], AF.Exp, scale=-0.5)
            return aggs, rs

        def ln_apply(x, aggs, rs, k, name):
            xh = dy3.tile([128, D], F32, tag=f"xh_{name}")
            nc.vector.tensor_scalar(xh[:], x[:], aggs[k][:, 0:1], rs[:, k:k + 1],
                                    OP.subtract, OP.mult)
            return xh

        def transpose_pair(xh_tiles, name, ones_row=True):
            xT = []
            for ch in range(2):
                t_ = dyn.tile([97, 256], F32, tag=f"xT{ch}", name=f"xT{ch}")
                for tt in range(2):
                    pt = pg(96, 128)
                    nc.tensor.transpose(pt[:], xh_tiles[tt][:, 96 * ch:96 * ch + 96],
                                        ident[:])
                    evac(t_[0:96, 128 * tt:128 * tt + 128], pt[:])
                if ones_row:
                    nc.gpsimd.memset(t_[96:97, :], 1.0)
                xT.append(t_)
            return xT

        # ================= stage 1: cross attention =================
        lstack1 = st.tile([32, 256], F32, tag="lstack1")
        av1 = []
        for j in range(NSLOT):
            kva = dyn.tile([96, KV[j]], F32, tag="kvTa", bufs=2)
            kvb = dyn.tile([97, KV[j]], F32, tag="kvTb", bufs=2)
            nc.sync.dma_start(kva[:], dr[f"kvT{j}"].ap()[0:96, :])
            nc.sync.dma_start(kvb[:], dr[f"kvT{j}"].ap()[96:193, :])
            qta = dyn.tile([96, 256], F32, tag="qTa")
            qtb = dyn.tile([97, 256], F32, tag="qTb")
            nc.sync.dma_start(qta[:], dr["qT"].ap()[193 * j:193 * j + 96, :])
            nc.sync.dma_start(qtb[:], dr["qT"].ap()[193 * j + 96:193 * j + 193, :])
            qsrc = [[(C[f"cwq{g}"][0][:], qta[:]), (C[f"cwq{g}"][1][:], qtb[:])]
                    for g in range(2)]
            ksrc = [[(C[f"cwk{g}"][0][:], kva[:]), (C[f"cwk{g}"][1][:], kvb[:])]
                    for g in range(2)]
            def vsrc(i, kva=kva, kvb=kvb):
                return [(kva[:, 128 * i:128 * i + 128], C["cwv"][0][:]),
                        (kvb[:, 128 * i:128 * i + 128], C["cwv"][1][:])]
            av_sb = attn_core(j, qsrc, ksrc, vsrc,
                              (dr[f"msk{j}"].ap()[:], 1), None,
                              KV[j], 25, lstack1, "s1")
            av1.append(av_sb)
        rstack1 = st.tile([32, 256], F32, tag="rstack1")
        nc.vector.reciprocal(rstack1[:], lstack1[:])
        r1 = []
        for j in range(NSLOT):
            r1.append(attn_finish(j, av1[j], rstack1, C["cwo"], None, "r1"))
        if debug:
            nc.sync.dma_start(dbg["d_avn"].ap()[:], av1[0][0][:])
            nc.sync.dma_start(dbg["d_r1"].ap()[:], r1[0][0][:])

        # ================= stages 2 (row) and 3 (col) =================
        def axial_stage(pre, res, res_tag, sname):
            x_all = [res[f][tt] for f in range(NSLOT) for tt in range(2)]
            aggs, rs = ln_stats(x_all, sname)
            lst = st.tile([32, 256], F32, tag=f"lstack_{sname}")
            avs = []
            for f in range(NSLOT):
                xh = [ln_apply(res[f][tt], aggs, rs, 2 * f + tt, sname)
                      for tt in range(2)]
                xT = transpose_pair(xh, sname)
                if debug and f == 0 and pre == "r":
                    nc.sync.dma_start(dbg["d_xh"].ap()[:], xh[0][:])
                    nc.gpsimd.dma_start(dbg["d_xT"].ap()[:], xT[0][:])
                qsrc = [[(C[f"{pre}wq{ch}{g}"][:], xT[ch][:]) for ch in range(2)]
                        for g in range(2)]
                ksrc = [[(C[f"{pre}wk{ch}{g}"][:], xT[ch][:]) for ch in range(2)]
                        for g in range(2)]
                def vsrc(i, xT=xT):
                    return [(xT[ch][0:97, 128 * i:128 * i + 128],
                             C[f"{pre}wv{ch}"][:]) for ch in range(2)]
                av_sb = attn_core(f, qsrc, ksrc, vsrc,
                                  (C[f"{pre}ka"][:], 3),
                                  C[f"{pre}qa"][:],
                                  NQ, 27, lst, pre)
                avs.append(av_sb)
            rst = st.tile([32, 256], F32, tag=f"rstack_{sname}")
            nc.vector.reciprocal(rst[:], lst[:])
            return [attn_finish(f, avs[f], rst, C[f"{pre}wo"], res[f], res_tag)
                    for f in range(NSLOT)]

        def dump_partial(res):
            for f in range(NSLOT):
                for tt in range(2):
                    row = 128 * (2 * f + tt)
                    nc.gpsimd.dma_start(out.ap()[row:row + 128, 0:D], res[f][tt][:])

        if stop_stage == 1:
            dump_partial(r1)
            r2 = None
        else:
            r2 = axial_stage("r", r1, "r2", "s2")
        if debug and r2 is not None:
            nc.sync.dma_start(dbg["d_r2"].ap()[:], r2[0][0][:])
        if stop_stage == 2 and r2 is not None:
            dump_partial(r2)
        r3 = axial_stage("l", r2, "r3", "s3") if stop_stage >= 3 else None
        if debug and r3 is not None:
            nc.sync.dma_start(dbg["d_r3"].ap()[:], r3[0][0][:])

        # ================= stage 4: FFN + head =================
        if stop_stage == 3 and r3 is not None:
            dump_partial(r3)
        x_all = [r3[f][tt] for f in range(NSLOT) for tt in range(2)] \
            if stop_stage >= 4 else []
        aggs4, rs4 = ln_stats(x_all, "s4") if stop_stage >= 4 else (None, None)
        for f in range(NSLOT if stop_stage >= 4 else 0):
            xh = [ln_apply(r3[f][tt], aggs4, rs4, 2 * f + tt, "s4")
                  for tt in range(2)]
            xT = transpose_pair(xh, "s4")
            h1g = []
            for q in range(6):
                ph = pg(128, 256)
                for ch in range(2):
                    nc.tensor.matmul(ph[:],
                                     C["fw1"][ch][:, 128 * q:128 * q + 128],
                                     xT[ch][:], start=(ch == 0), stop=(ch == 1))
                hg = dyn.tile([128, 256], F32, tag=f"h1g{q}")
                nc.scalar.activation(hg[:], ph[:], gelu_f)
                h1g.append(hg)
            if debug and f == 0:
                nc.gpsimd.dma_start(dbg["d_h1g"].ap()[:], h1g[0][:])
            z = []
            for tt in range(2):
                pz = pg(128, D)
                for q in range(6):
                    nc.tensor.matmul(pz[:], h1g[q][:, 128 * tt:128 * tt + 128],
                                     C["fw2"][:, D * q:D * q + D],
                                     start=(q == 0), stop=(q == 5))
                zt = dy3.tile([128, D], F32, tag="z_t")
                nc.vector.tensor_tensor(zt[:], pz[:], C["fb2"][:], OP.add)
                z_sb = st.tile([128, D], F32, tag=f"z_{f}_{tt}")
                nc.vector.tensor_tensor(z_sb[:], zt[:], r3[f][tt][:], OP.add)
                z.append(z_sb)
            if debug and f == 0:
                nc.sync.dma_start(dbg["d_z"].ap()[:], z[0][:])
            zT = transpose_pair(z, "hz", ones_row=False)
            for tt in range(2):
                po = pg(128, QL)
                for ch in range(2):
                    nc.tensor.matmul(po[:], zT[ch][0:96, 128 * tt:128 * tt + 128],
                                     C["hw"][:, QL * ch:QL * ch + QL],
                                     start=(ch == 0), stop=(ch == 1))
                ot = dy3.tile([128, QL], F32, tag="o_t")
                nc.vector.tensor_tensor(ot[:], po[:], C["hb"][:], OP.add)
                row = 128 * (2 * f + tt)
                nc.gpsimd.dma_start(out.ap()[row:row + 128, :], ot[:])

    nc.compile()
    _CACHE[key] = nc
    return nc


# ---------------------------------------------------------------- entry

def kernel(**inputs):
    inputs = {k: np.asarray(v, np.float32) for k, v in inputs.items()}
    nc = build_program()
    const = _host_constants(inputs)
    in_maps = [_core_inputs(inputs, const, c) for c in range(NCORE)]
    res = bass_utils.run_bass_kernel_spmd(nc, in_maps, core_ids=list(range(NCORE)))
    out = np.zeros((B, T - 1, H, W, QL), np.float32)
    for f in range(30):
        b, t = _frame(f)
        core, j = f % 8, f // 8
        o = res.results[core]["out"].reshape(NSLOT, 2 * 128, QL)
        out[b, t - 1] = o[j].reshape(H, W, QL)
    return out



# revision 9
# speedup vs baseline: 1.8130x; 1.8130x over previous
"""Trainium2 Bass kernel for nn_DecoderVectorized (axial decoder with causal
cross-attention). Self-contained: hardcodes all shapes/sharding.

Sharding: 32 SPMD slots = 8 cores x 4 slots over the B*(T-1)=30 frames
(sorted by t so slot j has kv prefix 128*{4,8,12,15}; 2 dummy slots).
"""
import math
import sys

import numpy as np

try:
    import concourse.bass as bass
except ImportError:  # pragma: no cover
    sys.path.insert(0, "/opt/trn_rl_repo")
    import concourse.bass as bass

import concourse.bacc as bacc
import concourse.mybir as mybir
import concourse.tile as tile
from concourse import bass_utils
from concourse.masks import make_identity

F32 = mybir.dt.float32
F32R = mybir.dt.float32r
BF16 = mybir.dt.bfloat16
import ml_dtypes
NPBF = ml_dtypes.bfloat16
AF = mybir.ActivationFunctionType
OP = mybir.AluOpType

H, W, D, HEADS, QL = 16, 16, 192, 8, 256
B, T, M = 2, 16, 128
NQ = H * W          # 256 tokens per frame
DH = D // HEADS     # 24
NCORE, NSLOT = 8, 4
TMAX = [4, 8, 12, 16]
KV = [t * 128 for t in TMAX]        # 512 1024 1536 2048
CM = 32.0                           # mask bias (power of 2: bf16-exact)
SCL = 1.0 / math.sqrt(DH)
EPS = 1e-5


def _frame(f):
    """frame index f in [0,32) -> (b, t); 30/31 are dummies."""
    if f >= 30:
        return (f - 30, 15)
    return (f % 2, f // 2 + 1)


# ---------------------------------------------------------------- host prep

def _qk_colmat_s1(w, bvec, g, is_q):
    """[193,128] colmat for stage-1 qhT/khT half g. Head h data at cols
    32h+1..32h+24 (aux row 0 first); col 32h+0: e_192 (ones) if is_q."""
    m = np.zeros((193, 128), np.float32)
    for h in range(4):
        Hh = 4 * g + h
        m[0:192, 32 * h + 1:32 * h + 25] = w[:, DH * Hh:DH * Hh + DH]
        m[192, 32 * h + 1:32 * h + 25] = bvec[DH * Hh:DH * Hh + DH]
        if is_q:
            m[192, 32 * h] = 1.0
    return m


def _qk_colmat_23(w, bvec, ch, g):
    """[97,128] colmat chunk for stage-2/3 qhT/khT. Data at cols 32h+3..+26;
    aux cols 0..2 zero (DMA'd)."""
    m = np.zeros((97, 128), np.float32)
    for h in range(4):
        Hh = 4 * g + h
        m[0:96, 32 * h + 3:32 * h + 27] = w[96 * ch:96 * ch + 96, DH * Hh:DH * Hh + DH]
        m[96, 32 * h + 3:32 * h + 27] = bvec[DH * Hh:DH * Hh + DH] * 0.5
    return m


def _wv_colmat_s1(w, bvec):
    """[193,256]: head Hh data at cols 32Hh..+23, ones col at 32Hh+24."""
    m = np.zeros((193, 256), np.float32)
    for Hh in range(8):
        m[0:192, 32 * Hh:32 * Hh + 24] = w[:, DH * Hh:DH * Hh + DH]
        m[192, 32 * Hh:32 * Hh + 24] = bvec[DH * Hh:DH * Hh + DH]
        m[192, 32 * Hh + 24] = 1.0
    return m


def _wv_colmat_23(w, bvec, ch):
    m = np.zeros((97, 256), np.float32)
    for Hh in range(8):
        m[0:96, 32 * Hh:32 * Hh + 24] = w[96 * ch:96 * ch + 96, DH * Hh:DH * Hh + DH]
        m[96, 32 * Hh:32 * Hh + 24] = bvec[DH * Hh:DH * Hh + DH] * 0.5
        m[96, 32 * Hh + 24] = 0.5
    return m


def _wo_aug(w, bvec):
    """[128, 384]: head H=4g+h at partition rows 32h..32h+31, col block 192g:
    rows 0..23 = wo rows, row 24 = bo/8 (walrus needs lhsT/rhs same base)."""
    m = np.zeros((128, 2 * D), np.float32)
    for g in range(2):
        for h in range(4):
            Hh = 4 * g + h
            m[32 * h:32 * h + 24, D * g:D * g + D] = w[DH * Hh:DH * Hh + DH, :]
            m[32 * h + 24, D * g:D * g + D] = bvec / 8.0
    return m


def _aux_rows(idx, is_q):
    """[128,256] aux contraction rows for the rank-3 axial mask, pre-spread to
    partition rows 32h+0..2. k-side: [ri^2, ri, 1]; q-side: [-c, 2c rj, -c rj^2]."""
    r = idx.astype(np.float32)
    if is_q:
        rows = np.stack([np.full(NQ, -CM, np.float32), 2.0 * CM * r, -CM * r * r])
    else:
        rows = np.stack([r * r, r, np.ones(NQ, np.float32)])
    m = np.zeros((128, NQ), np.float32)
    for h in range(4):
        m[32 * h:32 * h + 3] = rows
    return m


def _host_constants(inp):
    """Shared (core-independent) device constant arrays."""
    c = {}
    g, b_ = inp["rn_g"], inp["rn_b"]

    def eff(wq, bq, scale):
        return (g[:, None] * wq * scale).astype(np.float32), \
               ((b_ @ wq + bq) * scale).astype(np.float32)

    for gg in range(2):
        c[f"cwq{gg}"] = _qk_colmat_s1(inp["c_wq"] * SCL, inp["c_bq"] * SCL, gg, True)
        c[f"cwk{gg}"] = _qk_colmat_s1(inp["c_wk"], inp["c_bk"], gg, False)
    c["cwv"] = _wv_colmat_s1(inp["c_wv"], inp["c_bv"])
    c["cwo"] = _wo_aug(inp["c_wo"], inp["c_bo"])
    tok = np.arange(NQ)
    for pre, wp, idx in (("r", "r", tok // 16), ("l", "col", tok % 16)):
        wq, bq = eff(inp[wp + "_wq"], inp[wp + "_bq"], SCL)
        wk, bk = eff(inp[wp + "_wk"], inp[wp + "_bk"], 1.0)
        wv, bv = eff(inp[wp + "_wv"], inp[wp + "_bv"], 1.0)
        for ch in range(2):
            for gg in range(2):
                c[f"{pre}wq{ch}{gg}"] = _qk_colmat_23(wq, bq, ch, gg)
                c[f"{pre}wk{ch}{gg}"] = _qk_colmat_23(wk, bk, ch, gg)
            c[f"{pre}wv{ch}"] = _wv_colmat_23(wv, bv, ch)
        c[f"{pre}wo"] = _wo_aug(inp[wp + "_wo"], inp[wp + "_bo"])
        c[f"{pre}ka"] = _aux_rows(idx, False)
        c[f"{pre}qa"] = _aux_rows(idx, True)
    w1 = (inp["ff_ln_g"][:, None] * inp["ff_w1"]).astype(np.float32)
    b1 = (inp["ff_ln_b"] @ inp["ff_w1"] + inp["ff_b1"]).astype(np.float32)
    fw1 = np.zeros((2 * 97, 4 * D), np.float32)
    for ch in range(2):
        fw1[97 * ch:97 * ch + 96] = w1[96 * ch:96 * ch + 96]
        fw1[97 * ch + 96] = b1 * 0.5
    c["fw1"] = fw1
    fw2 = np.zeros((128, 6 * D), np.float32)
    for q in range(6):
        fw2[:, D * q:D * q + D] = inp["ff_w2"][128 * q:128 * q + 128, :]
    c["fw2"] = fw2
    c["fb2"] = np.broadcast_to(inp["ff_b2"][None], (128, D)).copy().astype(np.float32)
    hw = np.zeros((96, 2 * QL), np.float32)
    hw[:, 0:QL] = inp["head_w"][0:96]
    hw[:, QL:2 * QL] = inp["head_w"][96:192]
    c["hw"] = hw
    c["hb"] = np.broadcast_to(inp["head_b"][None], (128, QL)).copy().astype(np.float32)
    sel = np.zeros((128, 4), np.float32)
    for h in range(4):
        sel[32 * h + 24, h] = 1.0
    c["sel"] = sel
    rp = np.zeros((32, 8 * 128), np.float32)
    for k in range(8):
        for h in range(4):
            rp[4 * k + h, 128 * k + 32 * h:128 * k + 32 * h + 32] = 1.0
    c["repl"] = rp
    # everything feeding a matmul goes to bf16; fb2/hb stay f32 (DVE adds)
    for nm in list(c):
        if nm not in ("fb2", "hb"):
            c[nm] = c[nm].astype(NPBF)
    return c


def _core_inputs(inp, const, core):
    """Per-core in_map (includes the shared consts)."""
    m = dict(const)
    qg = np.asarray(inp["query_grid"], np.float32)
    tp = np.asarray(inp["t_pos_w"], np.float32)
    mt = np.asarray(inp["mem_tokens"], np.float32)
    qT = np.zeros((NSLOT * 193, NQ), np.float32)
    for j in range(NSLOT):
        b, t = _frame(8 * j + core)
        qT[193 * j:193 * j + 192] = (qg + tp[t][None, :]).T
        qT[193 * j + 192] = 1.0
        kvT = np.ones((193, KV[j]), np.float32)
        kvT[0:192] = mt[b, :TMAX[j]].reshape(-1, D).T
        m[f"kvT{j}"] = kvT.astype(NPBF)
        msk = np.zeros((4, KV[j]), np.float32)
        msk[:, 128 * t:] = -CM
        m[f"msk{j}"] = msk.astype(NPBF)
    m["qT"] = qT.astype(NPBF)
    return m


# ---------------------------------------------------------------- program

_CACHE = {}

# consts whose DRAM row-count exceeds 128: load as (rows0:97|0:96, rest) pairs
_SPLIT193 = ("cwq0", "cwq1", "cwk0", "cwk1", "cwv")


def build_program(gelu_f=AF.Gelu, debug=False, stop_stage=4):
    key = (gelu_f, debug, stop_stage)
    if key in _CACHE:
        return _CACHE[key]
    nc = bacc.Bacc("TRN2", target_bir_lowering=False, debug=False)

    # ---- DRAM I/O ----
    dr = {}
    def din(name, shape, dt=BF16):
        dr[name] = nc.dram_tensor(name, shape, dt, kind="ExternalInput")
    for gg in range(2):
        din(f"cwq{gg}", (193, 128)); din(f"cwk{gg}", (193, 128))
    din("cwv", (193, 256)); din("cwo", (128, 2 * D))
    for pre in ("r", "l"):
        for ch in range(2):
            for gg in range(2):
                din(f"{pre}wq{ch}{gg}", (97, 128)); din(f"{pre}wk{ch}{gg}", (97, 128))
            din(f"{pre}wv{ch}", (97, 256))
        din(f"{pre}wo", (128, 2 * D))
        din(f"{pre}ka", (128, NQ)); din(f"{pre}qa", (128, NQ))
    din("fw1", (2 * 97, 4 * D)); din("fw2", (128, 6 * D))
    din("fb2", (128, D), F32)
    din("hw", (96, 2 * QL)); din("hb", (128, QL), F32)
    din("sel", (128, 4)); din("repl", (32, 8 * 128))
    din("qT", (NSLOT * 193, NQ))
    for j in range(NSLOT):
        din(f"kvT{j}", (193, KV[j])); din(f"msk{j}", (4, KV[j]))
    out = nc.dram_tensor("out", (NSLOT * 2 * 128, QL), F32, kind="ExternalOutput")
    dbg = {}
    if debug:
        _BF = ("d_qhT", "d_khT", "d_pT", "d_av", "d_avn", "d_xT", "d_h1g")
        for nm, shape in (("d_qhT", (128, 256)), ("d_khT", (128, 512)),
                          ("d_pT", (128, 1024)), ("d_av", (128, 256)),
                          ("d_avn", (128, 256)), ("d_r1", (128, D)),
                          ("d_xh", (128, D)), ("d_xT", (97, 256)),
                          ("d_r2", (128, D)), ("d_r3", (128, D)),
                          ("d_h1g", (128, 256)), ("d_z", (128, D))):
            dbg[nm] = nc.dram_tensor(nm, shape, BF16 if nm in _BF else F32,
                                     kind="ExternalOutput")

    from contextlib import ExitStack
    with tile.TileContext(nc) as tc, ExitStack() as es:
        cst = es.enter_context(tc.tile_pool(name="cst", bufs=1))
        dyn = es.enter_context(tc.tile_pool(name="dyn", bufs=2))
        dy3 = es.enter_context(tc.tile_pool(name="dy3", bufs=3))
        dy8 = es.enter_context(tc.tile_pool(name="dy8", bufs=8))
        st = es.enter_context(tc.tile_pool(name="st", bufs=1))
        ps_s = es.enter_context(tc.tile_pool(name="ps_s", bufs=2, space="PSUM"))
        ps_a = es.enter_context(tc.tile_pool(name="ps_a", bufs=2, space="PSUM"))
        ps_g = es.enter_context(tc.tile_pool(name="ps_g", bufs=2, space="PSUM"))

        def mm(out, lhsT, rhs, **kw):
            # operands are bf16 tiles: 1 cyc/row on PE (vs fp32's 4)
            return nc.tensor.matmul(out, lhsT, rhs, **kw)

        def tp(out, in_, ident, **kw):
            return nc.tensor.matmul(out, in_, ident, is_transpose=True, **kw)

        def pg(p_, f_):
            return ps_g.tile([p_, f_], F32, tag="pg", name="pg")

        # ---- load constants ----
        C = {}
        for nm, t_ in dr.items():
            if nm == "qT" or nm.startswith(("kvT", "msk")):
                continue
            shape = list(t_.shape)
            dt_ = t_.dtype
            if nm in _SPLIT193:
                ta = cst.tile([96, shape[1]], dt_, tag=nm + "a")
                tb = cst.tile([97, shape[1]], dt_, tag=nm + "b")
                nc.sync.dma_start(ta[:], t_.ap()[0:96, :])
                nc.sync.dma_start(tb[:], t_.ap()[96:193, :])
                C[nm] = (ta, tb)
            elif nm == "fw1":
                ta = cst.tile([97, shape[1]], dt_, tag="fw1a")
                tb = cst.tile([97, shape[1]], dt_, tag="fw1b")
                nc.sync.dma_start(ta[:], t_.ap()[0:97, :])
                nc.sync.dma_start(tb[:], t_.ap()[97:194, :])
                C[nm] = (ta, tb)
            else:
                tl = cst.tile(shape, dt_, tag=nm)
                nc.sync.dma_start(tl[:], t_.ap()[:])
                C[nm] = tl
        ident = cst.tile([128, 128], F32, tag="ident")
        make_identity(nc, ident[:])
        epsc = cst.tile([128, 1], F32, tag="epsc")
        nc.gpsimd.memset(epsc[:], EPS)

        def evac(dst_ap, src_ap):
            nc.vector.tensor_copy(dst_ap, src_ap)

        # ============ generic attention core ============
        def attn_core(iid, qsrc, ksrc, vsrc, kauxdma, qauxdma, kvlen, kp,
                      lstack, pres):
            """qsrc/ksrc: per half, list of (lhsT_chunk, rhs_chunk) pairs.
            vsrc(i): chunk pairs for kv-tile i. kp = rows/head (25 or 27).
            Returns av_sb (unnormalized ovT + l rows) per half."""
            nkt = kvlen // 128
            qhT, khT = [], []
            for g in range(2):
                pq = pg(128, 256)
                for i, (cm, rhs) in enumerate(qsrc[g]):
                    mm(pq[:], cm, rhs, start=(i == 0),
                                     stop=(i == len(qsrc[g]) - 1))
                q_sb = dyn.tile([128, 256], BF16, tag=f"qhT{g}")
                evac(q_sb[:], pq[:])
                if qauxdma is not None:
                    for h in range(4):
                        nc.gpsimd.tensor_copy(q_sb[32 * h:32 * h + 3, :],
                                         qauxdma[32 * h:32 * h + 3, :])
                qhT.append(q_sb)
                k_sb = dyn.tile([128, kvlen], BF16, tag=f"khT{g}", bufs=1)
                for nchunk in range(0, kvlen, 512):
                    nw = min(512, kvlen - nchunk)
                    pk = pg(128, 512)
                    for i, (cm, rhs) in enumerate(ksrc[g]):
                        mm(pk[:, 0:nw], cm,
                                         rhs[:, nchunk:nchunk + nw],
                                         start=(i == 0),
                                         stop=(i == len(ksrc[g]) - 1))
                    evac(k_sb[:, nchunk:nchunk + nw], pk[:, 0:nw])
                if kauxdma[1] == 1:       # stage-1 dynamic mask row (DRAM)
                    for h in range(4):
                        nc.gpsimd.dma_start(k_sb[32 * h:32 * h + 1, :],
                                            kauxdma[0][h:h + 1, :])
                else:                      # static axial-mask rows (SBUF)
                    for h in range(4):
                        nc.gpsimd.tensor_copy(k_sb[32 * h:32 * h + 3, :],
                                         kauxdma[0][32 * h:32 * h + 3, :])
                khT.append(k_sb)
            av = [ps_a.tile([128, 256], F32, tag="p_av", name="p_av") for _ in range(2)]
            npair = nkt // 2
            for ip in range(npair):
                vhs = []
                for d in range(2):
                    pv = pg(128, 256)
                    vch = vsrc(2 * ip + d)
                    for ci, (cm, rhs) in enumerate(vch):
                        mm(pv[:], cm, rhs, start=(ci == 0),
                                         stop=(ci == len(vch) - 1))
                    vh = dy3.tile([128, 256], BF16, tag="vh")
                    evac(vh[:], pv[:])
                    vhs.append(vh)
                for g in range(2):
                    # head h in its own PSUM bank (cols 512h) -- concurrent
                    # row-group matmuls to one bank collide fatally on HW
                    pssT = ps_s.tile([128, 2048], F32, tag="p_sT", bufs=1)
                    for d in range(2):
                        for h in range(4):
                            mm(
                                pssT[:, 512 * h + 256 * d:512 * h + 256 * d + 256],
                                khT[g][32 * h:32 * h + kp,
                                       128 * (2 * ip + d):128 * (2 * ip + d) + 128],
                                qhT[g][32 * h:32 * h + kp, :],
                                start=True, stop=True, tile_position=(32 * h, 0))
                    pT = dy3.tile([128, 2048], BF16, tag="pT", bufs=2)
                    nc.scalar.activation(pT[:], pssT[:], AF.Exp)
                    if debug and iid == 0 and ip == 0 and g == 0 and pres == "s1":
                        nc.gpsimd.dma_start(dbg["d_pT"].ap()[:], pT[:, 0:1024])
                    for d in range(2):
                        for h in range(4):
                            mm(
                                av[g][32 * h:32 * h + 32, :],
                                vhs[d][:, 32 * (4 * g + h):32 * (4 * g + h) + 32],
                                pT[:, 512 * h + 256 * d:512 * h + 256 * d + 256],
                                start=(ip == 0 and d == 0),
                                stop=(ip == npair - 1 and d == 1),
                                tile_position=(0, 32 * h), skip_group_check=True)
            av_sb = []
            for g in range(2):
                a_sb = dy8.tile([128, 256], BF16, tag="av_sb")
                evac(a_sb[:], av[g][:])
                pl = pg(4, 256)
                mm(pl[:], C["sel"][:, 0:4], a_sb[:],
                                 start=True, stop=True)
                ltmp = dy3.tile([4, 256], F32, tag="ltmp")
                evac(ltmp[:], pl[:])
                nc.gpsimd.dma_start(lstack[8 * iid + 4 * g:8 * iid + 4 * g + 4, :],
                                    ltmp[:])
                av_sb.append(a_sb)
            if debug and iid == 0 and pres == "s1":
                nc.gpsimd.dma_start(dbg["d_qhT"].ap()[:], qhT[0][:])
                nc.gpsimd.dma_start(dbg["d_khT"].ap()[:], khT[0][:, 0:512])
                nc.sync.dma_start(dbg["d_av"].ap()[:], av_sb[0][:])
            return av_sb

        def attn_finish(iid, av_sb, rstack, wo_t, res_in, res_tag):
            """r-broadcast, normalize, wo projection (+residual)."""
            avn = []
            for g in range(2):
                prb = pg(128, 256)
                k = 2 * iid + g
                mm(prb[:], C["repl"][:, 128 * k:128 * k + 128],
                                 rstack[:], start=True, stop=True)
                an = dy3.tile([128, 256], BF16, tag="avn")
                nc.vector.tensor_tensor(an[:], av_sb[g][:], prb[:], OP.mult)
                avn.append(an)
            outs = []
            for tt in range(2):
                py = pg(128, D)
                for g in range(2):
                    mm(
                        py[:], avn[g][:, 128 * tt:128 * tt + 128],
                        wo_t[:, D * g:D * g + D],
                        start=(g == 0), stop=(g == 1))
                r_new = st.tile([128, D], F32, tag=f"{res_tag}_{iid}_{tt}")
                if res_in is None:
                    evac(r_new[:], py[:])
                else:
                    nc.vector.tensor_tensor(r_new[:], res_in[tt][:], py[:], OP.add)
                outs.append(r_new)
            return outs

        # ============ LN helpers ============
        def ln_stats(x_tiles, name):
            # tensor_tensor_reduce is fatal at runtime on this stack; use
            # bn_stats/bn_aggr (one DVE pass -> mean, var per partition)
            n = len(x_tiles)
            var = dyn.tile([128, n], F32, tag=f"var_{name}")
            rs = dyn.tile([128, n], F32, tag=f"rs_{name}")
            aggs = []
            for k, xt in enumerate(x_tiles):
                bst = dy3.tile([128, 6], F32, tag="bst")
                nc.vector.bn_stats(bst[:], xt[:])
                agg = dy8.tile([128, 2], F32, tag=f"agg_{name}", name="agg")
                nc.vector.bn_aggr(agg[:], bst[:])
                nc.vector.tensor_copy(var[:, k:k + 1], agg[:, 1:2])
                aggs.append(agg)
            lnv = dyn.tile([128, n], F32, tag=f"lnv_{name}")
            nc.scalar.activation(lnv[:], var[:], AF.Ln, bias=epsc[:])
            nc.scalar.activation(rs[:], lnv[:], AF.Exp, scale=-0.5)
            return aggs, rs

        def ln_apply(x, aggs, rs, k, name):
            xh = dy3.tile([128, D], F32, tag=f"xh_{name}")
            nc.vector.tensor_scalar(xh[:], x[:], aggs[k][:, 0:1], rs[:, k:k + 1],
                                    OP.subtract, OP.mult)
            return xh

        def transpose_pair(xh_tiles, name, ones_row=True):
            xT = []
            for ch in range(2):
                t_ = dyn.tile([97, 256], BF16, tag=f"xT{ch}", name=f"xT{ch}")
                for tt in range(2):
                    pt = pg(96, 128)
                    tp(pt[:], xh_tiles[tt][:, 96 * ch:96 * ch + 96],
                                        ident[:])
                    evac(t_[0:96, 128 * tt:128 * tt + 128], pt[:])
                if ones_row:
                    nc.gpsimd.memset(t_[96:97, :], 1.0)
                xT.append(t_)
            return xT

        # ================= stage 1: cross attention =================
        lstack1 = st.tile([32, 256], F32, tag="lstack1")
        av1 = []
        for j in range(NSLOT):
            kva = dyn.tile([96, KV[j]], BF16, tag="kvTa", bufs=2)
            kvb = dyn.tile([97, KV[j]], BF16, tag="kvTb", bufs=2)
            nc.sync.dma_start(kva[:], dr[f"kvT{j}"].ap()[0:96, :])
            nc.sync.dma_start(kvb[:], dr[f"kvT{j}"].ap()[96:193, :])
            qta = dyn.tile([96, 256], BF16, tag="qTa")
            qtb = dyn.tile([97, 256], BF16, tag="qTb")
            nc.sync.dma_start(qta[:], dr["qT"].ap()[193 * j:193 * j + 96, :])
            nc.sync.dma_start(qtb[:], dr["qT"].ap()[193 * j + 96:193 * j + 193, :])
            qsrc = [[(C[f"cwq{g}"][0][:], qta[:]), (C[f"cwq{g}"][1][:], qtb[:])]
                    for g in range(2)]
            ksrc = [[(C[f"cwk{g}"][0][:], kva[:]), (C[f"cwk{g}"][1][:], kvb[:])]
                    for g in range(2)]
            def vsrc(i, kva=kva, kvb=kvb):
                return [(kva[:, 128 * i:128 * i + 128], C["cwv"][0][:]),
                        (kvb[:, 128 * i:128 * i + 128], C["cwv"][1][:])]
            av_sb = attn_core(j, qsrc, ksrc, vsrc,
                              (dr[f"msk{j}"].ap()[:], 1), None,
                              KV[j], 25, lstack1, "s1")
            av1.append(av_sb)
        rstack1 = st.tile([32, 256], BF16, tag="rstack1")
        with nc.allow_low_precision("1/l in bf16: |rel| ~4e-3 acceptable"):
            nc.vector.reciprocal(rstack1[:], lstack1[:])
        r1 = []
        for j in range(NSLOT):
            r1.append(attn_finish(j, av1[j], rstack1, C["cwo"], None, "r1"))
        if debug:
            nc.sync.dma_start(dbg["d_avn"].ap()[:], av1[0][0][:])
            nc.sync.dma_start(dbg["d_r1"].ap()[:], r1[0][0][:])

        # ================= stages 2 (row) and 3 (col) =================
        def axial_stage(pre, res, res_tag, sname):
            x_all = [res[f][tt] for f in range(NSLOT) for tt in range(2)]
            aggs, rs = ln_stats(x_all, sname)
            lst = st.tile([32, 256], F32, tag=f"lstack_{sname}")
            avs = []
            for f in range(NSLOT):
                xh = [ln_apply(res[f][tt], aggs, rs, 2 * f + tt, sname)
                      for tt in range(2)]
                xT = transpose_pair(xh, sname)
                if debug and f == 0 and pre == "r":
                    nc.sync.dma_start(dbg["d_xh"].ap()[:], xh[0][:])
                    nc.gpsimd.dma_start(dbg["d_xT"].ap()[:], xT[0][:])
                qsrc = [[(C[f"{pre}wq{ch}{g}"][:], xT[ch][:]) for ch in range(2)]
                        for g in range(2)]
                ksrc = [[(C[f"{pre}wk{ch}{g}"][:], xT[ch][:]) for ch in range(2)]
                        for g in range(2)]
                def vsrc(i, xT=xT):
                    return [(xT[ch][0:97, 128 * i:128 * i + 128],
                             C[f"{pre}wv{ch}"][:]) for ch in range(2)]
                av_sb = attn_core(f, qsrc, ksrc, vsrc,
                                  (C[f"{pre}ka"][:], 3),
                                  C[f"{pre}qa"][:],
                                  NQ, 27, lst, pre)
                avs.append(av_sb)
            rst = st.tile([32, 256], BF16, tag=f"rstack_{sname}")
            with nc.allow_low_precision("1/l in bf16"):
                nc.vector.reciprocal(rst[:], lst[:])
            return [attn_finish(f, avs[f], rst, C[f"{pre}wo"], res[f], res_tag)
                    for f in range(NSLOT)]

        def dump_partial(res):
            for f in range(NSLOT):
                for tt in range(2):
                    row = 128 * (2 * f + tt)
                    nc.gpsimd.dma_start(out.ap()[row:row + 128, 0:D], res[f][tt][:])

        if stop_stage == 1:
            dump_partial(r1)
            r2 = None
        else:
            r2 = axial_stage("r", r1, "r2", "s2")
        if debug and r2 is not None:
            nc.sync.dma_start(dbg["d_r2"].ap()[:], r2[0][0][:])
        if stop_stage == 2 and r2 is not None:
            dump_partial(r2)
        r3 = axial_stage("l", r2, "r3", "s3") if stop_stage >= 3 else None
        if debug and r3 is not None:
            nc.sync.dma_start(dbg["d_r3"].ap()[:], r3[0][0][:])

        # ================= stage 4: FFN + head =================
        if stop_stage == 3 and r3 is not None:
            dump_partial(r3)
        x_all = [r3[f][tt] for f in range(NSLOT) for tt in range(2)] \
            if stop_stage >= 4 else []
        aggs4, rs4 = ln_stats(x_all, "s4") if stop_stage >= 4 else (None, None)
        for f in range(NSLOT if stop_stage >= 4 else 0):
            xh = [ln_apply(r3[f][tt], aggs4, rs4, 2 * f + tt, "s4")
                  for tt in range(2)]
            xT = transpose_pair(xh, "s4")
            h1g = []
            for q in range(6):
                ph = pg(128, 256)
                for ch in range(2):
                    mm(ph[:],
                                     C["fw1"][ch][:, 128 * q:128 * q + 128],
                                     xT[ch][:], start=(ch == 0), stop=(ch == 1))
                hg = dyn.tile([128, 256], BF16, tag=f"h1g{q}")
                nc.scalar.activation(hg[:], ph[:], gelu_f)
                h1g.append(hg)
            if debug and f == 0:
                nc.gpsimd.dma_start(dbg["d_h1g"].ap()[:], h1g[0][:])
            z = []
            for tt in range(2):
                pz = pg(128, D)
                for q in range(6):
                    mm(pz[:], h1g[q][:, 128 * tt:128 * tt + 128],
                                     C["fw2"][:, D * q:D * q + D],
                                     start=(q == 0), stop=(q == 5))
                zt = dy3.tile([128, D], F32, tag="z_t")
                nc.vector.tensor_tensor(zt[:], pz[:], C["fb2"][:], OP.add)
                z_sb = st.tile([128, D], F32, tag=f"z_{f}_{tt}")
                nc.vector.tensor_tensor(z_sb[:], zt[:], r3[f][tt][:], OP.add)
                z.append(z_sb)
            if debug and f == 0:
                nc.sync.dma_start(dbg["d_z"].ap()[:], z[0][:])
            zT = transpose_pair(z, "hz", ones_row=False)
            for tt in range(2):
                po = pg(128, QL)
                for ch in range(2):
                    mm(po[:], zT[ch][0:96, 128 * tt:128 * tt + 128],
                                     C["hw"][:, QL * ch:QL * ch + QL],
                                     start=(ch == 0), stop=(ch == 1))
                ot = dy3.tile([128, QL], F32, tag="o_t")
                nc.vector.tensor_tensor(ot[:], po[:], C["hb"][:], OP.add)
                row = 128 * (2 * f + tt)
                nc.gpsimd.dma_start(out.ap()[row:row + 128, :], ot[:])

    nc.compile()
    _CACHE[key] = nc
    return nc


# ---------------------------------------------------------------- entry

def kernel(**inputs):
    inputs = {k: np.asarray(v, np.float32) for k, v in inputs.items()}
    nc = build_program()
    const = _host_constants(inputs)
    in_maps = [_core_inputs(inputs, const, c) for c in range(NCORE)]
    res = bass_utils.run_bass_kernel_spmd(nc, in_maps, core_ids=list(range(NCORE)))
    out = np.zeros((B, T - 1, H, W, QL), np.float32)
    for f in range(30):
        b, t = _frame(f)
        core, j = f % 8, f // 8
        o = res.results[core]["out"].reshape(NSLOT, 2 * 128, QL)
        out[b, t - 1] = o[j].reshape(H, W, QL)
    return out



# revision 14
# speedup vs baseline: 1.9686x; 1.0858x over previous
"""Trainium2 Bass kernel for nn_DecoderVectorized (axial decoder with causal
cross-attention). Self-contained: hardcodes all shapes/sharding.

Sharding: 32 SPMD slots = 8 cores x 4 slots over the B*(T-1)=30 frames
(sorted by t so slot j has kv prefix 128*{4,8,12,15}; 2 dummy slots).
"""
import math
import sys

import numpy as np

try:
    import concourse.bass as bass
except ImportError:  # pragma: no cover
    sys.path.insert(0, "/opt/trn_rl_repo")
    import concourse.bass as bass

import concourse.bacc as bacc
import concourse.mybir as mybir
import concourse.tile as tile
from concourse import bass_utils
from concourse.masks import make_identity

F32 = mybir.dt.float32
F32R = mybir.dt.float32r
BF16 = mybir.dt.bfloat16
import ml_dtypes
NPBF = ml_dtypes.bfloat16
AF = mybir.ActivationFunctionType
OP = mybir.AluOpType

H, W, D, HEADS, QL = 16, 16, 192, 8, 256
B, T, M = 2, 16, 128
NQ = H * W          # 256 tokens per frame
DH = D // HEADS     # 24
NCORE, NSLOT = 8, 4
TMAX = [4, 8, 12, 16]
KV = [t * 128 for t in TMAX]        # 512 1024 1536 2048
CM = 32.0                           # mask bias (power of 2: bf16-exact)
SCL = 1.0 / math.sqrt(DH)
EPS = 1e-5


def _frame(f):
    """frame index f in [0,32) -> (b, t); 30/31 are dummies."""
    if f >= 30:
        return (f - 30, 15)
    return (f % 2, f // 2 + 1)


# ---------------------------------------------------------------- host prep

def _k_colmat_s1(w, bvec, g):
    """[193,128] colmat for stage-1 khT half g; head h data at cols
    32h+0..23 (stage-1 mask is tile-aligned -> exp bias, no aux rows)."""
    m = np.zeros((193, 128), np.float32)
    for h in range(4):
        Hh = 4 * g + h
        m[0:192, 32 * h:32 * h + 24] = w[:, DH * Hh:DH * Hh + DH]
        m[192, 32 * h:32 * h + 24] = bvec[DH * Hh:DH * Hh + DH]
    return m


def _qk_colmat_23(w, bvec, ch, g):
    """[97,128] colmat chunk for stage-2/3 qhT/khT. Data at cols 32h+3..+26;
    aux cols 0..2 zero (DMA'd)."""
    m = np.zeros((97, 128), np.float32)
    for h in range(4):
        Hh = 4 * g + h
        m[0:96, 32 * h + 3:32 * h + 27] = w[96 * ch:96 * ch + 96, DH * Hh:DH * Hh + DH]
        m[96, 32 * h + 3:32 * h + 27] = bvec[DH * Hh:DH * Hh + DH] * 0.5
    return m


def _wv_colmat_s1(w, bvec):
    """[193,256]: head Hh: l/ones col at 32Hh+0 (32-aligned for the per-g
    softmax finish), data at cols 32Hh+1..+24."""
    m = np.zeros((193, 256), np.float32)
    for Hh in range(8):
        m[0:192, 32 * Hh + 1:32 * Hh + 25] = w[:, DH * Hh:DH * Hh + DH]
        m[192, 32 * Hh + 1:32 * Hh + 25] = bvec[DH * Hh:DH * Hh + DH]
        m[192, 32 * Hh] = 1.0
    return m


def _wv_colmat_23(w, bvec, ch):
    m = np.zeros((97, 256), np.float32)
    for Hh in range(8):
        m[0:96, 32 * Hh + 1:32 * Hh + 25] = w[96 * ch:96 * ch + 96, DH * Hh:DH * Hh + DH]
        m[96, 32 * Hh + 1:32 * Hh + 25] = bvec[DH * Hh:DH * Hh + DH] * 0.5
        m[96, 32 * Hh] = 0.5
    return m


def _wo_aug(w, bvec):
    """[128, 384]: head H=4g+h at partition rows 32h..32h+31, col block 192g:
    row 0 = bo/8 (the avn l-row is 1 after normalize), rows 1..24 = wo rows."""
    m = np.zeros((128, 2 * D), np.float32)
    for g in range(2):
        for h in range(4):
            Hh = 4 * g + h
            m[32 * h + 1:32 * h + 25, D * g:D * g + D] = w[DH * Hh:DH * Hh + DH, :]
            m[32 * h, D * g:D * g + D] = bvec / 8.0
    return m


def _aux_rows(idx, is_q):
    """[128,256] aux contraction rows for the rank-3 axial mask, pre-spread to
    partition rows 32h+0..2. k-side: [ri^2, ri, 1]; q-side: [-c, 2c rj, -c rj^2]."""
    r = idx.astype(np.float32)
    if is_q:
        rows = np.stack([np.full(NQ, -CM, np.float32), 2.0 * CM * r, -CM * r * r])
    else:
        rows = np.stack([r * r, r, np.ones(NQ, np.float32)])
    m = np.zeros((128, NQ), np.float32)
    for h in range(4):
        m[32 * h:32 * h + 3] = rows
    return m


def _host_constants(inp):
    """Shared (core-independent) device constant arrays."""
    c = {}
    g, b_ = inp["rn_g"], inp["rn_b"]

    def eff(wq, bq, scale):
        return (g[:, None] * wq * scale).astype(np.float32), \
               ((b_ @ wq + bq) * scale).astype(np.float32)

    for gg in range(2):
        c[f"cwk{gg}"] = _k_colmat_s1(inp["c_wk"], inp["c_bk"], gg)
    c["cwv"] = _wv_colmat_s1(inp["c_wv"], inp["c_bv"])
    c["cwo"] = _wo_aug(inp["c_wo"], inp["c_bo"])
    tok = np.arange(NQ)
    for pre, wp, idx in (("r", "r", tok // 16), ("l", "col", tok % 16)):
        wq, bq = eff(inp[wp + "_wq"], inp[wp + "_bq"], SCL)
        wk, bk = eff(inp[wp + "_wk"], inp[wp + "_bk"], 1.0)
        wv, bv = eff(inp[wp + "_wv"], inp[wp + "_bv"], 1.0)
        for ch in range(2):
            for gg in range(2):
                c[f"{pre}wq{ch}{gg}"] = _qk_colmat_23(wq, bq, ch, gg)
                c[f"{pre}wk{ch}{gg}"] = _qk_colmat_23(wk, bk, ch, gg)
            c[f"{pre}wv{ch}"] = _wv_colmat_23(wv, bv, ch)
        c[f"{pre}wo"] = _wo_aug(inp[wp + "_wo"], inp[wp + "_bo"])
        c[f"{pre}ka"] = _aux_rows(idx, False)
        c[f"{pre}qa"] = _aux_rows(idx, True)
    w1 = (inp["ff_ln_g"][:, None] * inp["ff_w1"]).astype(np.float32)
    b1 = (inp["ff_ln_b"] @ inp["ff_w1"] + inp["ff_b1"]).astype(np.float32)
    fw1 = np.zeros((2 * 97, 4 * D), np.float32)
    for ch in range(2):
        fw1[97 * ch:97 * ch + 96] = w1[96 * ch:96 * ch + 96]
        fw1[97 * ch + 96] = b1 * 0.5
    c["fw1"] = fw1
    fw2 = np.zeros((128, 6 * D), np.float32)
    for q in range(6):
        fw2[:, D * q:D * q + D] = inp["ff_w2"][128 * q:128 * q + 128, :]
    c["fw2"] = fw2
    c["fb2"] = np.broadcast_to(inp["ff_b2"][None], (128, D)).copy().astype(np.float32)
    hw = np.zeros((96, 2 * QL), np.float32)
    hw[:, 0:QL] = inp["head_w"][0:96]
    hw[:, QL:2 * QL] = inp["head_w"][96:192]
    c["hw"] = hw
    c["hb"] = np.broadcast_to(inp["head_b"][None], (128, QL)).copy().astype(np.float32)
    rp4 = np.zeros((128, 128), np.float32)
    for h in range(4):
        rp4[32 * h, 32 * h:32 * h + 32] = 1.0
    c["rp4"] = rp4
    # everything feeding a matmul goes to bf16; fb2/hb stay f32 (DVE adds)
    for nm in list(c):
        if nm not in ("fb2", "hb"):
            c[nm] = c[nm].astype(NPBF)
    return c


NKT = [t for t in TMAX]                 # k-tiles per slot: 4 8 12 16
CSTART = [0, 4, 12, 24]                 # mskc col offset per slot (cumsum)


def _core_inputs(inp, const, core):
    """Per-core in_map (includes the shared consts)."""
    m = dict(const)
    qg = np.asarray(inp["query_grid"], np.float32)
    tp = np.asarray(inp["t_pos_w"], np.float32)
    mt = np.asarray(inp["mem_tokens"], np.float32)
    wq = np.asarray(inp["c_wq"], np.float32) * SCL
    bq = np.asarray(inp["c_bq"], np.float32) * SCL
    # host-computed stage-1 qhT: [128, NSLOT*2*256], (j,g) block at 512j+256g;
    # head h data at partition 32h+0..23
    qhT = np.zeros((128, NSLOT * 2 * 256), np.float32)
    # per-(slot, k-tile) additive mask bias column (tile-aligned causal mask)
    mskc = np.zeros((128, sum(NKT)), np.float32)
    b0, t0 = _frame(core)
    kvT = np.ones((193, 2048), np.float32)
    kvT[0:192] = mt[b0].reshape(-1, D).T
    m["kvT"] = kvT.astype(NPBF)
    for j in range(NSLOT):
        b, t = _frame(8 * j + core)
        assert b == b0, "slots of one core share the batch index"
        qh = (qg + tp[t][None, :]) @ wq + bq[None, :]     # [256, 192]
        for g in range(2):
            for h in range(4):
                Hh = 4 * g + h
                qhT[32 * h:32 * h + 24, 512 * j + 256 * g:512 * j + 256 * g + 256] =                     qh[:, DH * Hh:DH * Hh + DH].T
        mskc[:, CSTART[j] + t:CSTART[j] + NKT[j]] = -1e9
    m["qhT"] = qhT.astype(NPBF)
    m["mskc"] = mskc
    return m


# ---------------------------------------------------------------- program

_CACHE = {}

# consts whose DRAM row-count exceeds 128: load as (rows0:97|0:96, rest) pairs
_SPLIT193 = ("cwk0", "cwk1", "cwv", "kvT")


def build_program(gelu_f=AF.Gelu, debug=False, stop_stage=4):
    key = (gelu_f, debug, stop_stage)
    if key in _CACHE:
        return _CACHE[key]
    nc = bacc.Bacc("TRN2", target_bir_lowering=False, debug=False)

    # ---- DRAM I/O ----
    dr = {}
    def din(name, shape, dt=BF16):
        dr[name] = nc.dram_tensor(name, shape, dt, kind="ExternalInput")
    for gg in range(2):
        din(f"cwk{gg}", (193, 128))
    din("cwv", (193, 256)); din("cwo", (128, 2 * D))
    for pre in ("r", "l"):
        for ch in range(2):
            for gg in range(2):
                din(f"{pre}wq{ch}{gg}", (97, 128)); din(f"{pre}wk{ch}{gg}", (97, 128))
            din(f"{pre}wv{ch}", (97, 256))
        din(f"{pre}wo", (128, 2 * D))
        din(f"{pre}ka", (128, NQ)); din(f"{pre}qa", (128, NQ))
    din("fw1", (2 * 97, 4 * D)); din("fw2", (128, 6 * D))
    din("fb2", (128, D), F32)
    din("hw", (96, 2 * QL)); din("hb", (128, QL), F32)
    din("rp4", (128, 128))
    din("kvT", (193, 2048))
    din("qhT", (128, NSLOT * 2 * 256))
    din("mskc", (128, sum(NKT)), F32)
    out = nc.dram_tensor("out", (NSLOT * 2 * 128, QL), F32, kind="ExternalOutput")
    dbg = {}
    if debug:
        _BF = ("d_xT", "d_h1g")
        for nm, shape in (("d_r1", (128, D)),
                          ("d_xh", (128, D)), ("d_xT", (97, 256)),
                          ("d_r2", (128, D)), ("d_r3", (128, D)),
                          ("d_h1g", (128, 256)), ("d_z", (128, D))):
            dbg[nm] = nc.dram_tensor(nm, shape, BF16 if nm in _BF else F32,
                                     kind="ExternalOutput")

    from contextlib import ExitStack
    with tile.TileContext(nc) as tc, ExitStack() as es:
        cst = es.enter_context(tc.tile_pool(name="cst", bufs=1))
        dyn = es.enter_context(tc.tile_pool(name="dyn", bufs=2))
        dy3 = es.enter_context(tc.tile_pool(name="dy3", bufs=3))
        dy8 = es.enter_context(tc.tile_pool(name="dy8", bufs=8))
        st = es.enter_context(tc.tile_pool(name="st", bufs=1))
        ps_s = es.enter_context(tc.tile_pool(name="ps_s", bufs=2, space="PSUM"))
        ps_a = es.enter_context(tc.tile_pool(name="ps_a", bufs=2, space="PSUM"))
        ps_g = es.enter_context(tc.tile_pool(name="ps_g", bufs=2, space="PSUM"))

        def mm(out, lhsT, rhs, **kw):
            # operands are bf16 tiles: 1 cyc/row on PE (vs fp32's 4)
            return nc.tensor.matmul(out, lhsT, rhs, **kw)

        def tp(out, in_, ident, **kw):
            return nc.tensor.matmul(out, in_, ident, is_transpose=True, **kw)

        def pg(p_, f_):
            return ps_g.tile([p_, f_], F32, tag="pg", name="pg")

        # ---- load constants ----
        C = {}
        for nm, t_ in dr.items():
            shape = list(t_.shape)
            dt_ = t_.dtype
            if nm in _SPLIT193:
                ta = cst.tile([96, shape[1]], dt_, tag=nm + "a")
                tb = cst.tile([97, shape[1]], dt_, tag=nm + "b")
                nc.sync.dma_start(ta[:], t_.ap()[0:96, :])
                nc.sync.dma_start(tb[:], t_.ap()[96:193, :])
                C[nm] = (ta, tb)
            elif nm == "fw1":
                ta = cst.tile([97, shape[1]], dt_, tag="fw1a")
                tb = cst.tile([97, shape[1]], dt_, tag="fw1b")
                nc.sync.dma_start(ta[:], t_.ap()[0:97, :])
                nc.sync.dma_start(tb[:], t_.ap()[97:194, :])
                C[nm] = (ta, tb)
            else:
                tl = cst.tile(shape, dt_, tag=nm)
                nc.sync.dma_start(tl[:], t_.ap()[:])
                C[nm] = tl
        ident = cst.tile([128, 128], F32, tag="ident")
        make_identity(nc, ident[:])
        epsc = cst.tile([128, 1], F32, tag="epsc")
        nc.gpsimd.memset(epsc[:], EPS)

        def evac(dst_ap, src_ap):
            nc.vector.tensor_copy(dst_ap, src_ap)

        # ============ generic attention core ============
        def score_av(qh, kh, vv, nkt, bias, pres):
            """qh(g,h)->AP [kp,256]; kh(g,h,d)->AP [kp,128]; vv(d)->AP [128,256]
            (l/ones col at 32Hh+0, data +1..24); bias(d)->AP [128,1] or None.
            Per-d exp (strided) overlaps Act with the next d's scores on PE.
            Returns the two [128,256] PSUM accumulators (unnormalized ovT)."""
            av = [ps_a.tile([128, 256], F32, tag="p_av", name="p_av")
                  for _ in range(2)]
            npair = nkt // 2
            for ip in range(npair):
                for g in range(2):
                    # head h in its own PSUM bank (cols 512h) -- concurrent
                    # row-group matmuls to one bank collide fatally on HW
                    pssT = ps_s.tile([128, 2048], F32, tag="p_sT", bufs=1)
                    pT = dy3.tile([128, 2048], BF16, tag="pT", bufs=2)
                    psr = pssT[:].rearrange("p (h x) -> p h x", h=4)
                    ptr = pT[:].rearrange("p (h x) -> p h x", h=4)
                    for dd in range(2):
                        d = 2 * ip + dd
                        for h in range(4):
                            mm(pssT[:, 512 * h + 256 * dd:512 * h + 256 * dd + 256],
                               kh(g, h, d), qh(g, h),
                               start=True, stop=True, tile_position=(32 * h, 0))
                        nc.scalar.activation(
                            ptr[:, :, 256 * dd:256 * dd + 256],
                            psr[:, :, 256 * dd:256 * dd + 256], AF.Exp,
                            bias=bias(d) if bias is not None else 0.0)
                    for dd in range(2):
                        d = 2 * ip + dd
                        for h in range(4):
                            mm(av[g][32 * h:32 * h + 32, :],
                               vv(d)[:, 32 * (4 * g + h):32 * (4 * g + h) + 32],
                               pT[:, 512 * h + 256 * dd:512 * h + 256 * dd + 256],
                               start=(ip == 0 and dd == 0),
                               stop=(ip == npair - 1 and dd == 1),
                               tile_position=(0, 32 * h), skip_group_check=True)
            return av

        def attn_finish(iid, av_ps, wo_t, res_in, res_tag):
            """Per-half: evac av, broadcast l (rows 32h) across its band via
            rp4, normalize by 1/l, then wo projection (+residual)."""
            avn = []
            for g in range(2):
                a_sb = dy8.tile([128, 256], BF16, tag="av_sb")
                evac(a_sb[:], av_ps[g][:])
                plb = pg(128, 256)
                mm(plb[:], C["rp4"][:], a_sb[:], start=True, stop=True)
                rec = dy3.tile([128, 256], BF16, tag="rec")
                with nc.allow_low_precision("1/l in bf16: ~4e-3 rel ok"):
                    nc.vector.reciprocal(rec[:], plb[:])
                an = dy3.tile([128, 256], BF16, tag="avn")
                nc.vector.tensor_tensor(an[:], a_sb[:], rec[:], OP.mult)
                avn.append(an)
            outs = []
            for tt in range(2):
                py = pg(128, D)
                for g in range(2):
                    mm(py[:], avn[g][:, 128 * tt:128 * tt + 128],
                       wo_t[:, D * g:D * g + D],
                       start=(g == 0), stop=(g == 1))
                r_new = st.tile([128, D], F32, tag=f"{res_tag}_{iid}_{tt}")
                if res_in is None:
                    evac(r_new[:], py[:])
                else:
                    nc.vector.tensor_tensor(r_new[:], res_in[tt][:], py[:],
                                            OP.add)
                outs.append(r_new)
            return outs

        # ============ LN helpers (per-frame: no cross-frame barrier) ============
        def ln_pair(xa, xb, name):
            var = dyn.tile([128, 2], F32, tag="ln_var", name=f"var_{name}")
            rs = dyn.tile([128, 2], F32, tag="ln_rs", name=f"rs_{name}")
            aggs = []
            for k, xt in enumerate((xa, xb)):
                bst = dy3.tile([128, 6], F32, tag="bst")
                nc.vector.bn_stats(bst[:], xt[:])
                agg = dy8.tile([128, 2], F32, tag="ln_agg", name=f"agg_{name}")
                nc.vector.bn_aggr(agg[:], bst[:])
                nc.vector.tensor_copy(var[:, k:k + 1], agg[:, 1:2])
                aggs.append(agg)
            lnv = dyn.tile([128, 2], F32, tag="ln_lnv", name=f"lnv_{name}")
            nc.scalar.activation(lnv[:], var[:], AF.Ln, bias=epsc[:])
            nc.scalar.activation(rs[:], lnv[:], AF.Exp, scale=-0.5)
            return aggs, rs

        def ln_apply(x, agg, rs, k, name):
            xh = dy3.tile([128, D], F32, tag=f"xh_{name}")
            nc.vector.tensor_scalar(xh[:], x[:], agg[:, 0:1], rs[:, k:k + 1],
                                    OP.subtract, OP.mult)
            return xh

        def transpose_pair(xh_tiles, name, ones_row=True):
            xT = []
            for ch in range(2):
                t_ = dyn.tile([97, 256], BF16, tag=f"xT{ch}", name=f"xT{ch}",
                              bufs=3)
                for tt in range(2):
                    pt = pg(96, 128)
                    tp(pt[:], xh_tiles[tt][:, 96 * ch:96 * ch + 96],
                       ident[:])
                    evac(t_[0:96, 128 * tt:128 * tt + 128], pt[:])
                if ones_row:
                    nc.gpsimd.memset(t_[96:97, :], 1.0)
                xT.append(t_)
            return xT

        # ================= stage 1: cross attention =================
        # K/V projections are shared across the 4 slots (their kv prefixes
        # nest); the causal mask is tile-aligned -> per-(slot,tile) exp bias.
        kva, kvb = C["kvT"]
        qh1 = C["qhT"]
        k_all = []
        for g in range(2):
            ka = st.tile([128, 2048], BF16, tag=f"kall{g}")
            for c4 in range(4):
                pk = pg(128, 512)
                mm(pk[:], C[f"cwk{g}"][0][:], kva[:, 512 * c4:512 * c4 + 512],
                   start=True, stop=False)
                mm(pk[:], C[f"cwk{g}"][1][:], kvb[:, 512 * c4:512 * c4 + 512],
                   start=False, stop=True)
                evac(ka[:, 512 * c4:512 * c4 + 512], pk[:])
            k_all.append(ka)
        v_all = []
        for d in range(16):
            pv = pg(128, 256)
            mm(pv[:], kva[:, 128 * d:128 * d + 128], C["cwv"][0][:],
               start=True, stop=False)
            mm(pv[:], kvb[:, 128 * d:128 * d + 128], C["cwv"][1][:],
               start=False, stop=True)
            vt = st.tile([128, 256], BF16, tag=f"vall{d}")
            evac(vt[:], pv[:])
            v_all.append(vt)

        r1 = []
        for j in range(NSLOT):
            def qh(g, h, j=j):
                return qh1[32 * h:32 * h + 24,
                           512 * j + 256 * g:512 * j + 256 * g + 256]
            def kh(g, h, d):
                return k_all[g][32 * h:32 * h + 24, 128 * d:128 * d + 128]
            def vv(d):
                return v_all[d][:]
            def bias(d, j=j):
                return C["mskc"][:, CSTART[j] + d:CSTART[j] + d + 1]
            av = score_av(qh, kh, vv, NKT[j], bias, "s1")
            r1.append(attn_finish(j, av, C["cwo"], None, "r1"))
        if debug:
            nc.sync.dma_start(dbg["d_r1"].ap()[:], r1[0][0][:])

        # ================= stages 2 (row) and 3 (col) =================
        def axial_stage(pre, res, res_tag, sname):
            outs = []
            for f in range(NSLOT):
                aggs, rs = ln_pair(res[f][0], res[f][1], f"{sname}{f}")
                xh = [ln_apply(res[f][tt], aggs[tt], rs, tt, sname)
                      for tt in range(2)]
                xT = transpose_pair(xh, sname)
                if debug and f == 0 and pre == "r":
                    nc.sync.dma_start(dbg["d_xh"].ap()[:], xh[0][:])
                    nc.sync.dma_start(dbg["d_xT"].ap()[:], xT[0][:])
                qkT = {}
                for which, aux in (("wq", "qa"), ("wk", "ka")):
                    for g in range(2):
                        pq = pg(128, 256)
                        mm(pq[:], C[f"{pre}{which}0{g}"][:], xT[0][:],
                           start=True, stop=False)
                        mm(pq[:], C[f"{pre}{which}1{g}"][:], xT[1][:],
                           start=False, stop=True)
                        sb = dyn.tile([128, 256], BF16,
                                      tag=f"{which}hT{g}", bufs=2)
                        evac(sb[:], pq[:])
                        for h in range(4):
                            nc.gpsimd.tensor_copy(
                                sb[32 * h:32 * h + 3, :],
                                C[f"{pre}{aux}"][32 * h:32 * h + 3, :])
                        qkT[(which, g)] = sb
                vhs = []
                for d in range(2):
                    pv = pg(128, 256)
                    mm(pv[:], xT[0][0:97, 128 * d:128 * d + 128],
                       C[f"{pre}wv0"][:], start=True, stop=False)
                    mm(pv[:], xT[1][0:97, 128 * d:128 * d + 128],
                       C[f"{pre}wv1"][:], start=False, stop=True)
                    vt = dy3.tile([128, 256], BF16, tag="vh")
                    evac(vt[:], pv[:])
                    vhs.append(vt)
                def qh(g, h, qkT=qkT):
                    return qkT[("wq", g)][32 * h:32 * h + 27, :]
                def kh(g, h, d, qkT=qkT):
                    return qkT[("wk", g)][32 * h:32 * h + 27,
                                          128 * d:128 * d + 128]
                def vv(d, vhs=vhs):
                    return vhs[d][:]
                av = score_av(qh, kh, vv, 2, None, pre)
                outs.append(attn_finish(f, av, C[f"{pre}wo"], res[f], res_tag))
            return outs

        def dump_partial(res):
            for f in range(NSLOT):
                for tt in range(2):
                    row = 128 * (2 * f + tt)
                    nc.sync.dma_start(out.ap()[row:row + 128, 0:D], res[f][tt][:])

        if stop_stage == 1:
            dump_partial(r1)
            r2 = None
        else:
            r2 = axial_stage("r", r1, "r2", "s2")
        if debug and r2 is not None:
            nc.sync.dma_start(dbg["d_r2"].ap()[:], r2[0][0][:])
        if stop_stage == 2 and r2 is not None:
            dump_partial(r2)
        r3 = axial_stage("l", r2, "r3", "s3") if stop_stage >= 3 else None
        if debug and r3 is not None:
            nc.sync.dma_start(dbg["d_r3"].ap()[:], r3[0][0][:])

        # ================= stage 4: FFN + head =================
        if stop_stage == 3 and r3 is not None:
            dump_partial(r3)
        for f in range(NSLOT if stop_stage >= 4 else 0):
            aggs4, rs4 = ln_pair(r3[f][0], r3[f][1], f"s4{f}")
            xh = [ln_apply(r3[f][tt], aggs4[tt], rs4, tt, "s4")
                  for tt in range(2)]
            xT = transpose_pair(xh, "s4")
            h1g = []
            for q in range(6):
                ph = pg(128, 256)
                for ch in range(2):
                    mm(ph[:],
                                     C["fw1"][ch][:, 128 * q:128 * q + 128],
                                     xT[ch][:], start=(ch == 0), stop=(ch == 1))
                hg = dyn.tile([128, 256], BF16, tag=f"h1g{q}")
                nc.scalar.activation(hg[:], ph[:], gelu_f)
                h1g.append(hg)
            if debug and f == 0:
                nc.sync.dma_start(dbg["d_h1g"].ap()[:], h1g[0][:])
            z = []
            for tt in range(2):
                pz = pg(128, D)
                for q in range(6):
                    mm(pz[:], h1g[q][:, 128 * tt:128 * tt + 128],
                                     C["fw2"][:, D * q:D * q + D],
                                     start=(q == 0), stop=(q == 5))
                zt = dy3.tile([128, D], F32, tag="z_t")
                nc.vector.tensor_tensor(zt[:], pz[:], C["fb2"][:], OP.add)
                z_sb = st.tile([128, D], F32, tag=f"z_{f}_{tt}")
                nc.vector.tensor_tensor(z_sb[:], zt[:], r3[f][tt][:], OP.add)
                z.append(z_sb)
            if debug and f == 0:
                nc.sync.dma_start(dbg["d_z"].ap()[:], z[0][:])
            zT = transpose_pair(z, "hz", ones_row=False)
            for tt in range(2):
                po = pg(128, QL)
                for ch in range(2):
                    mm(po[:], zT[ch][0:96, 128 * tt:128 * tt + 128],
                                     C["hw"][:, QL * ch:QL * ch + QL],
                                     start=(ch == 0), stop=(ch == 1))
                ot = dy3.tile([128, QL], F32, tag="o_t")
                nc.vector.tensor_tensor(ot[:], po[:], C["hb"][:], OP.add)
                row = 128 * (2 * f + tt)
                nc.sync.dma_start(out.ap()[row:row + 128, :], ot[:])

    nc.compile()
    _CACHE[key] = nc
    return nc


# ---------------------------------------------------------------- entry

def kernel(**inputs):
    inputs = {k: np.asarray(v, np.float32) for k, v in inputs.items()}
    nc = build_program()
    const = _host_constants(inputs)
    in_maps = [_core_inputs(inputs, const, c) for c in range(NCORE)]
    res = bass_utils.run_bass_kernel_spmd(nc, in_maps, core_ids=list(range(NCORE)))
    out = np.zeros((B, T - 1, H, W, QL), np.float32)
    for f in range(30):
        b, t = _frame(f)
        core, j = f % 8, f // 8
        o = res.results[core]["out"].reshape(NSLOT, 2 * 128, QL)
        out[b, t - 1] = o[j].reshape(H, W, QL)
    return out



# revision 18
# speedup vs baseline: 3.1982x; 1.6246x over previous
"""Trainium2 Bass kernel for nn_DecoderVectorized (axial decoder with causal
cross-attention). Self-contained: hardcodes all shapes/sharding.

Sharding: 32 SPMD slots = 8 cores x 4 slots over the B*(T-1)=30 frames
(sorted by t so slot j has kv prefix 128*{4,8,12,15}; 2 dummy slots).
"""
import math
import sys

import numpy as np

try:
    import concourse.bass as bass
except ImportError:  # pragma: no cover
    sys.path.insert(0, "/opt/trn_rl_repo")
    import concourse.bass as bass

import concourse.bacc as bacc
import concourse.mybir as mybir
import concourse.tile as tile
from concourse import bass_utils
from concourse.masks import make_identity

F32 = mybir.dt.float32
F32R = mybir.dt.float32r
BF16 = mybir.dt.bfloat16
import ml_dtypes
NPBF = ml_dtypes.bfloat16
AF = mybir.ActivationFunctionType
OP = mybir.AluOpType

H, W, D, HEADS, QL = 16, 16, 192, 8, 256
B, T, M = 2, 16, 128
NQ = H * W          # 256 tokens per frame
DH = D // HEADS     # 24
NCORE, NSLOT = 8, 4
TMAX = [4, 8, 12, 16]
KV = [t * 128 for t in TMAX]        # 512 1024 1536 2048
CM = 32.0                           # mask bias (power of 2: bf16-exact)
SCL = 1.0 / math.sqrt(DH)
EPS = 1e-5


def _frame(f):
    """frame index f in [0,32) -> (b, t); 30/31 are dummies."""
    if f >= 30:
        return (f - 30, 15)
    return (f % 2, f // 2 + 1)


# ---------------------------------------------------------------- host prep

def _k_colmat_s1(w, bvec, g):
    """[193,128] colmat for stage-1 khT half g; head h data at cols
    32h+0..23 (stage-1 mask is tile-aligned -> exp bias, no aux rows)."""
    m = np.zeros((193, 128), np.float32)
    for h in range(4):
        Hh = 4 * g + h
        m[0:192, 32 * h:32 * h + 24] = w[:, DH * Hh:DH * Hh + DH]
        m[192, 32 * h:32 * h + 24] = bvec[DH * Hh:DH * Hh + DH]
    return m


def _qk_colmat_23(w, bvec, ch, g):
    """[97,128] colmat chunk for stage-2/3 qhT/khT. Data at cols 32h+3..+26;
    aux cols 0..2 zero (DMA'd)."""
    m = np.zeros((97, 128), np.float32)
    for h in range(4):
        Hh = 4 * g + h
        m[0:96, 32 * h + 3:32 * h + 27] = w[96 * ch:96 * ch + 96, DH * Hh:DH * Hh + DH]
        m[96, 32 * h + 3:32 * h + 27] = bvec[DH * Hh:DH * Hh + DH] * 0.5
    return m


def _wv_colmat_s1(w, bvec):
    """[193,256]: head Hh: l/ones col at 32Hh+0 (32-aligned for the per-g
    softmax finish), data at cols 32Hh+1..+24."""
    m = np.zeros((193, 256), np.float32)
    for Hh in range(8):
        m[0:192, 32 * Hh + 1:32 * Hh + 25] = w[:, DH * Hh:DH * Hh + DH]
        m[192, 32 * Hh + 1:32 * Hh + 25] = bvec[DH * Hh:DH * Hh + DH]
        m[192, 32 * Hh] = 1.0
    return m


def _wv_colmat_23(w, bvec, ch):
    m = np.zeros((97, 256), np.float32)
    for Hh in range(8):
        m[0:96, 32 * Hh + 1:32 * Hh + 25] = w[96 * ch:96 * ch + 96, DH * Hh:DH * Hh + DH]
        m[96, 32 * Hh + 1:32 * Hh + 25] = bvec[DH * Hh:DH * Hh + DH] * 0.5
        m[96, 32 * Hh] = 0.5
    return m


def _wo_aug(w, bvec):
    """[128, 384]: head H=4g+h at partition rows 32h..32h+31, col block 192g:
    row 0 = bo/8 (the avn l-row is 1 after normalize), rows 1..24 = wo rows."""
    m = np.zeros((128, 2 * D), np.float32)
    for g in range(2):
        for h in range(4):
            Hh = 4 * g + h
            m[32 * h + 1:32 * h + 25, D * g:D * g + D] = w[DH * Hh:DH * Hh + DH, :]
            m[32 * h, D * g:D * g + D] = bvec / 8.0
    return m


def _aux_rows(idx, is_q):
    """[128,256] aux contraction rows for the rank-3 axial mask, pre-spread to
    partition rows 32h+0..2. k-side: [ri^2, ri, 1]; q-side: [-c, 2c rj, -c rj^2]."""
    r = idx.astype(np.float32)
    if is_q:
        rows = np.stack([np.full(NQ, -CM, np.float32), 2.0 * CM * r, -CM * r * r])
    else:
        rows = np.stack([r * r, r, np.ones(NQ, np.float32)])
    m = np.zeros((128, NQ), np.float32)
    for h in range(4):
        m[32 * h:32 * h + 3] = rows
    return m


def _host_constants(inp):
    """Shared (core-independent) device constant arrays."""
    c = {}
    g, b_ = inp["rn_g"], inp["rn_b"]

    def eff(wq, bq, scale):
        return (g[:, None] * wq * scale).astype(np.float32), \
               ((b_ @ wq + bq) * scale).astype(np.float32)

    for gg in range(2):
        c[f"cwk{gg}"] = _k_colmat_s1(inp["c_wk"], inp["c_bk"], gg)
    c["cwv"] = _wv_colmat_s1(inp["c_wv"], inp["c_bv"])
    c["cwo"] = _wo_aug(inp["c_wo"], inp["c_bo"])
    tok = np.arange(NQ)
    for pre, wp, idx in (("r", "r", tok // 16), ("l", "col", tok % 16)):
        wq, bq = eff(inp[wp + "_wq"], inp[wp + "_bq"], SCL)
        wk, bk = eff(inp[wp + "_wk"], inp[wp + "_bk"], 1.0)
        wv, bv = eff(inp[wp + "_wv"], inp[wp + "_bv"], 1.0)
        for ch in range(2):
            for gg in range(2):
                c[f"{pre}wq{ch}{gg}"] = _qk_colmat_23(wq, bq, ch, gg)
                c[f"{pre}wk{ch}{gg}"] = _qk_colmat_23(wk, bk, ch, gg)
            c[f"{pre}wv{ch}"] = _wv_colmat_23(wv, bv, ch)
        c[f"{pre}wo"] = _wo_aug(inp[wp + "_wo"], inp[wp + "_bo"])
        c[f"{pre}ka"] = _aux_rows(idx, False)
        c[f"{pre}qa"] = _aux_rows(idx, True)
    w1 = (inp["ff_ln_g"][:, None] * inp["ff_w1"]).astype(np.float32)
    b1 = (inp["ff_ln_b"] @ inp["ff_w1"] + inp["ff_b1"]).astype(np.float32)
    fw1 = np.zeros((2 * 97, 4 * D), np.float32)
    for ch in range(2):
        fw1[97 * ch:97 * ch + 96] = w1[96 * ch:96 * ch + 96]
        fw1[97 * ch + 96] = b1 * 0.5
    c["fw1"] = fw1
    fw2 = np.zeros((128, 6 * D), np.float32)
    for q in range(6):
        fw2[:, D * q:D * q + D] = inp["ff_w2"][128 * q:128 * q + 128, :]
    c["fw2"] = fw2
    c["fb2"] = np.broadcast_to(inp["ff_b2"][None], (128, D)).copy().astype(np.float32)
    hw = np.zeros((96, 2 * QL), np.float32)
    hw[:, 0:QL] = inp["head_w"][0:96]
    hw[:, QL:2 * QL] = inp["head_w"][96:192]
    c["hw"] = hw
    c["hb"] = np.broadcast_to(inp["head_b"][None], (128, QL)).copy().astype(np.float32)
    rp4 = np.zeros((128, 128), np.float32)
    for h in range(4):
        rp4[32 * h, 32 * h:32 * h + 32] = 1.0
    c["rp4"] = rp4
    # everything feeding a matmul goes to bf16; fb2/hb stay f32 (DVE adds)
    for nm in list(c):
        if nm not in ("fb2", "hb"):
            c[nm] = c[nm].astype(NPBF)
    return c


NKT = [t for t in TMAX]                 # k-tiles per slot: 4 8 12 16
CSTART = [0, 4, 12, 24]                 # mskc col offset per slot (cumsum)


def _core_inputs(inp, const, core):
    """Per-core in_map (includes the shared consts)."""
    m = dict(const)
    qg = np.asarray(inp["query_grid"], np.float32)
    tp = np.asarray(inp["t_pos_w"], np.float32)
    mt = np.asarray(inp["mem_tokens"], np.float32)
    wq = np.asarray(inp["c_wq"], np.float32) * SCL
    bq = np.asarray(inp["c_bq"], np.float32) * SCL
    # host-computed stage-1 qhT: [128, NSLOT*2*256], (j,g) block at 512j+256g;
    # head h data at partition 32h+0..23
    qhT = np.zeros((128, NSLOT * 2 * 256), np.float32)
    # per-(slot, k-tile) multiplicative keep-column (1 keep / 0 masked),
    # applied post-exp as pT *= col; the causal mask is 128-tile aligned
    mskc = np.zeros((128, sum(NKT)), np.float32)
    b0, t0 = _frame(core)
    kvT = np.ones((193, 2048), np.float32)
    kvT[0:192] = mt[b0].reshape(-1, D).T
    m["kvT"] = kvT.astype(NPBF)
    for j in range(NSLOT):
        b, t = _frame(8 * j + core)
        assert b == b0, "slots of one core share the batch index"
        qh = (qg + tp[t][None, :]) @ wq + bq[None, :]     # [256, 192]
        for g in range(2):
            for h in range(4):
                Hh = 4 * g + h
                qhT[32 * h:32 * h + 24, 512 * j + 256 * g:512 * j + 256 * g + 256] =                     qh[:, DH * Hh:DH * Hh + DH].T
        mskc[:, CSTART[j]:CSTART[j] + NKT[j]] = 1.0
        mskc[:, CSTART[j] + t:CSTART[j] + NKT[j]] = 0.0
    m["qhT"] = qhT.astype(NPBF)
    m["mskc"] = mskc
    return m


# ---------------------------------------------------------------- program

_CACHE = {}

# consts whose DRAM row-count exceeds 128: load as (rows0:97|0:96, rest) pairs
_SPLIT193 = ("cwk0", "cwk1", "cwv", "kvT")


def build_program(gelu_f=AF.Gelu, debug=False, stop_stage=4):
    key = (gelu_f, debug, stop_stage)
    if key in _CACHE:
        return _CACHE[key]
    nc = bacc.Bacc("TRN2", target_bir_lowering=False, debug=False)

    # ---- DRAM I/O ----
    dr = {}
    def din(name, shape, dt=BF16):
        dr[name] = nc.dram_tensor(name, shape, dt, kind="ExternalInput")
    # dict order == DMA issue order: stage-1 inputs first (startup latency)
    din("kvT", (193, 2048))
    for gg in range(2):
        din(f"cwk{gg}", (193, 128))
    din("cwv", (193, 256)); din("qhT", (128, NSLOT * 2 * 256))
    din("mskc", (128, sum(NKT)), F32)
    din("rp4", (128, 128)); din("cwo", (128, 2 * D))
    for pre in ("r", "l"):
        for ch in range(2):
            for gg in range(2):
                din(f"{pre}wq{ch}{gg}", (97, 128)); din(f"{pre}wk{ch}{gg}", (97, 128))
            din(f"{pre}wv{ch}", (97, 256))
        din(f"{pre}wo", (128, 2 * D))
        din(f"{pre}ka", (128, NQ)); din(f"{pre}qa", (128, NQ))
    din("fw1", (2 * 97, 4 * D)); din("fw2", (128, 6 * D))
    din("fb2", (128, D), F32)
    din("hw", (96, 2 * QL)); din("hb", (128, QL), F32)
    out = nc.dram_tensor("out", (NSLOT * 2 * 128, QL), F32, kind="ExternalOutput")
    dbg = {}
    if debug:
        _BF = ("d_xT", "d_h1g")
        for nm, shape in (("d_r1", (128, D)),
                          ("d_xh", (128, D)), ("d_xT", (97, 256)),
                          ("d_r2", (128, D)), ("d_r3", (128, D)),
                          ("d_h1g", (128, 256)), ("d_z", (128, D))):
            dbg[nm] = nc.dram_tensor(nm, shape, BF16 if nm in _BF else F32,
                                     kind="ExternalOutput")

    from contextlib import ExitStack
    with tile.TileContext(nc) as tc, ExitStack() as es:
        cst = es.enter_context(tc.tile_pool(name="cst", bufs=1))
        dyn = es.enter_context(tc.tile_pool(name="dyn", bufs=2))
        dy3 = es.enter_context(tc.tile_pool(name="dy3", bufs=3))
        dy8 = es.enter_context(tc.tile_pool(name="dy8", bufs=8))
        st = es.enter_context(tc.tile_pool(name="st", bufs=1))
        ps_s = es.enter_context(tc.tile_pool(name="ps_s", bufs=2, space="PSUM"))
        ps_a = es.enter_context(tc.tile_pool(name="ps_a", bufs=2, space="PSUM"))
        ps_g = es.enter_context(tc.tile_pool(name="ps_g", bufs=2, space="PSUM"))

        def mm(out, lhsT, rhs, **kw):
            # operands are bf16 tiles: 1 cyc/row on PE (vs fp32's 4)
            return nc.tensor.matmul(out, lhsT, rhs, **kw)

        def tp(out, in_, ident, **kw):
            return nc.tensor.matmul(out, in_, ident, is_transpose=True, **kw)

        def pg(p_, f_):
            return ps_g.tile([p_, f_], F32, tag="pg", name="pg")

        # ---- load constants ----
        C = {}
        for nm, t_ in dr.items():
            shape = list(t_.shape)
            dt_ = t_.dtype
            if nm in _SPLIT193:
                ta = cst.tile([96, shape[1]], dt_, tag=nm + "a")
                tb = cst.tile([97, shape[1]], dt_, tag=nm + "b")
                nc.sync.dma_start(ta[:], t_.ap()[0:96, :])
                nc.sync.dma_start(tb[:], t_.ap()[96:193, :])
                C[nm] = (ta, tb)
            elif nm == "fw1":
                ta = cst.tile([97, shape[1]], dt_, tag="fw1a")
                tb = cst.tile([97, shape[1]], dt_, tag="fw1b")
                nc.sync.dma_start(ta[:], t_.ap()[0:97, :])
                nc.sync.dma_start(tb[:], t_.ap()[97:194, :])
                C[nm] = (ta, tb)
            else:
                tl = cst.tile(shape, dt_, tag=nm)
                nc.sync.dma_start(tl[:], t_.ap()[:])
                C[nm] = tl
        ident = cst.tile([128, 128], F32, tag="ident")
        make_identity(nc, ident[:])
        epsc = cst.tile([128, 1], F32, tag="epsc")
        nc.gpsimd.memset(epsc[:], EPS)

        def evac(dst_ap, src_ap):
            nc.vector.tensor_copy(dst_ap, src_ap)

        # ============ generic attention core ============
        def score_av(qh, kh, vv, nkt, mz, pres):
            """qh(g,h)->AP [kp,256]; kh(g,h,d)->AP [kp,128]; vv(d)->AP [128,256]
            (l/ones col at 32Hh+0, data +1..24); mz(d) -> multiplicative
            keep-column AP [128,1] (stage-1 causal mask) or None.
            Scores go to 2-bank head-pair slabs (head 2hp+hh in bank hh) with
            bufs=2, so Act exps slab n while PE fills slab n+1 -- including
            across the g halves. Returns the two [128,256] PSUM accumulators."""
            av = [ps_a.tile([128, 256], F32, tag="p_av", name="p_av")
                  for _ in range(2)]
            npair = nkt // 2
            for ip in range(npair):
                for g in range(2):
                    for hp in range(2):
                        slab = ps_s.tile([128, 1024], F32, tag="p_sT", bufs=2)
                        pTs = dy3.tile([128, 1024], BF16, tag="pT", bufs=4)
                        for dd in range(2):
                            d = 2 * ip + dd
                            for hh in range(2):
                                h = 2 * hp + hh
                                mm(slab[:, 512 * hh + 256 * dd:
                                        512 * hh + 256 * dd + 256],
                                   kh(g, h, d), qh(g, h),
                                   start=True, stop=True,
                                   tile_position=(32 * h, 0))
                        nc.scalar.activation(pTs[:], slab[:], AF.Exp)
                        if mz is not None:
                            ptr = pTs[:].rearrange("p (a x) -> p a x", a=2)
                            for dd in range(2):
                                d = 2 * ip + dd
                                col = mz(d)
                                if col is None:
                                    continue
                                sl = ptr[:, :, 256 * dd:256 * dd + 256]
                                nc.vector.tensor_scalar(sl, sl, col, None,
                                                        OP.mult)
                        for dd in range(2):
                            d = 2 * ip + dd
                            for hh in range(2):
                                h = 2 * hp + hh
                                mm(av[g][32 * h:32 * h + 32, :],
                                   vv(d)[:, 32 * (4 * g + h):
                                         32 * (4 * g + h) + 32],
                                   pTs[:, 512 * hh + 256 * dd:
                                       512 * hh + 256 * dd + 256],
                                   start=(ip == 0 and dd == 0),
                                   stop=(ip == npair - 1 and dd == 1),
                                   tile_position=(0, 32 * h),
                                   skip_group_check=True)
            return av

        def attn_finish(iid, av_ps, wo_t, res_in, res_tag):
            """Per-half: evac av, broadcast l (rows 32h) across its band via
            rp4, normalize by 1/l, then wo projection (+residual)."""
            avn = []
            for g in range(2):
                a_sb = dy8.tile([128, 256], BF16, tag="av_sb")
                evac(a_sb[:], av_ps[g][:])
                plb = pg(128, 256)
                mm(plb[:], C["rp4"][:], a_sb[:], start=True, stop=True)
                rec = dy3.tile([128, 256], BF16, tag="rec")
                with nc.allow_low_precision("1/l in bf16: ~4e-3 rel ok"):
                    nc.vector.reciprocal(rec[:], plb[:])
                an = dy3.tile([128, 256], BF16, tag="avn")
                nc.vector.tensor_tensor(an[:], a_sb[:], rec[:], OP.mult)
                avn.append(an)
            outs = []
            for tt in range(2):
                py = pg(128, D)
                for g in range(2):
                    mm(py[:], avn[g][:, 128 * tt:128 * tt + 128],
                       wo_t[:, D * g:D * g + D],
                       start=(g == 0), stop=(g == 1))
                r_new = st.tile([128, D], F32, tag=f"{res_tag}_{iid}_{tt}")
                if res_in is None:
                    evac(r_new[:], py[:])
                else:
                    nc.vector.tensor_tensor(r_new[:], res_in[tt][:], py[:],
                                            OP.add)
                outs.append(r_new)
            return outs

        # ============ LN helpers (per-frame: no cross-frame barrier) ============
        def ln_pair(xa, xb, name):
            var = dyn.tile([128, 2], F32, tag="ln_var", name=f"var_{name}")
            rs = dyn.tile([128, 2], F32, tag="ln_rs", name=f"rs_{name}")
            aggs = []
            for k, xt in enumerate((xa, xb)):
                bst = dy3.tile([128, 6], F32, tag="bst")
                nc.vector.bn_stats(bst[:], xt[:])
                agg = dy8.tile([128, 2], F32, tag="ln_agg", name=f"agg_{name}")
                nc.vector.bn_aggr(agg[:], bst[:])
                nc.vector.tensor_copy(var[:, k:k + 1], agg[:, 1:2])
                aggs.append(agg)
            lnv = dyn.tile([128, 2], F32, tag="ln_lnv", name=f"lnv_{name}")
            nc.scalar.activation(lnv[:], var[:], AF.Ln, bias=epsc[:])
            nc.scalar.activation(rs[:], lnv[:], AF.Exp, scale=-0.5)
            return aggs, rs

        def ln_apply(x, agg, rs, k, name):
            xh = dy3.tile([128, D], F32, tag=f"xh_{name}")
            nc.vector.tensor_scalar(xh[:], x[:], agg[:, 0:1], rs[:, k:k + 1],
                                    OP.subtract, OP.mult)
            return xh

        def transpose_pair(xh_tiles, name, ones_row=True, bufs=5):
            xT = []
            for ch in range(2):
                t_ = dyn.tile([97, 256], BF16, tag=f"xT{ch}", name=f"xT{ch}",
                              bufs=bufs)
                for tt in range(2):
                    pt = pg(96, 128)
                    tp(pt[:], xh_tiles[tt][:, 96 * ch:96 * ch + 96],
                       ident[:])
                    evac(t_[0:96, 128 * tt:128 * tt + 128], pt[:])
                if ones_row:
                    nc.gpsimd.memset(t_[96:97, :], 1.0)
                xT.append(t_)
            return xT

        # ================= stage 1: cross attention =================
        # K/V projections are shared across the 4 slots (their kv prefixes
        # nest); the causal mask is tile-aligned -> per-(slot,tile) exp bias.
        kva, kvb = C["kvT"]
        qh1 = C["qhT"]
        k_all = []
        for g in range(2):
            ka = st.tile([128, 2048], BF16, tag=f"kall{g}")
            for c4 in range(4):
                pk = pg(128, 512)
                mm(pk[:], C[f"cwk{g}"][0][:], kva[:, 512 * c4:512 * c4 + 512],
                   start=True, stop=False)
                mm(pk[:], C[f"cwk{g}"][1][:], kvb[:, 512 * c4:512 * c4 + 512],
                   start=False, stop=True)
                evac(ka[:, 512 * c4:512 * c4 + 512], pk[:])
            k_all.append(ka)
        v_all = []
        for d in range(16):
            pv = pg(128, 256)
            mm(pv[:], kva[:, 128 * d:128 * d + 128], C["cwv"][0][:],
               start=True, stop=False)
            mm(pv[:], kvb[:, 128 * d:128 * d + 128], C["cwv"][1][:],
               start=False, stop=True)
            vt = st.tile([128, 256], BF16, tag=f"vall{d}")
            evac(vt[:], pv[:])
            v_all.append(vt)

        r1 = []
        for j in range(NSLOT):
            def qh(g, h, j=j):
                return qh1[32 * h:32 * h + 24,
                           512 * j + 256 * g:512 * j + 256 * g + 256]
            def kh(g, h, d):
                return k_all[g][32 * h:32 * h + 24, 128 * d:128 * d + 128]
            def vv(d):
                return v_all[d][:]
            def mz(d, j=j):
                # tiles below 4j+1 are unmasked for every core (t > 4j)
                if d < 4 * j + 1:
                    return None
                return C["mskc"][:, CSTART[j] + d:CSTART[j] + d + 1]
            av = score_av(qh, kh, vv, NKT[j], mz, "s1")
            r1.append(attn_finish(j, av, C["cwo"], None, "r1"))
        if debug:
            nc.sync.dma_start(dbg["d_r1"].ap()[:], r1[0][0][:])

        # ================= stages 2 (row) and 3 (col) =================
        # Phased across frames (A: LN+transpose, B: q/k/v, C: attention) so
        # each in-order engine queue sees all frames of a phase back-to-back
        # and cross-frame pipelining hides the per-frame latency chain.
        def axial_stage(pre, res, res_tag, sname):
            xTs = []
            for f in range(NSLOT):
                aggs, rs = ln_pair(res[f][0], res[f][1], f"{sname}{f}")
                xh = [ln_apply(res[f][tt], aggs[tt], rs, tt, sname)
                      for tt in range(2)]
                xT = transpose_pair(xh, sname)
                if debug and f == 0 and pre == "r":
                    nc.sync.dma_start(dbg["d_xh"].ap()[:], xh[0][:])
                    nc.sync.dma_start(dbg["d_xT"].ap()[:], xT[0][:])
                xTs.append(xT)
            qk, vs = [], []
            for f in range(NSLOT):
                xT = xTs[f]
                qkT = {}
                for which, aux in (("wq", "qa"), ("wk", "ka")):
                    for g in range(2):
                        pq = pg(128, 256)
                        mm(pq[:], C[f"{pre}{which}0{g}"][:], xT[0][:],
                           start=True, stop=False)
                        mm(pq[:], C[f"{pre}{which}1{g}"][:], xT[1][:],
                           start=False, stop=True)
                        sb = dyn.tile([128, 256], BF16,
                                      tag=f"{which}hT{g}", bufs=5)
                        evac(sb[:], pq[:])
                        for h in range(4):
                            nc.gpsimd.tensor_copy(
                                sb[32 * h:32 * h + 3, :],
                                C[f"{pre}{aux}"][32 * h:32 * h + 3, :])
                        qkT[(which, g)] = sb
                vhs = []
                for d in range(2):
                    pv = pg(128, 256)
                    mm(pv[:], xT[0][0:97, 128 * d:128 * d + 128],
                       C[f"{pre}wv0"][:], start=True, stop=False)
                    mm(pv[:], xT[1][0:97, 128 * d:128 * d + 128],
                       C[f"{pre}wv1"][:], start=False, stop=True)
                    vt = dy3.tile([128, 256], BF16, tag="vh", bufs=10)
                    evac(vt[:], pv[:])
                    vhs.append(vt)
                qk.append(qkT)
                vs.append(vhs)
            outs = []
            for f in range(NSLOT):
                qkT, vhs = qk[f], vs[f]
                def qh(g, h, qkT=qkT):
                    return qkT[("wq", g)][32 * h:32 * h + 27, :]
                def kh(g, h, d, qkT=qkT):
                    return qkT[("wk", g)][32 * h:32 * h + 27,
                                          128 * d:128 * d + 128]
                def vv(d, vhs=vhs):
                    return vhs[d][:]
                av = score_av(qh, kh, vv, 2, None, pre)
                outs.append(attn_finish(f, av, C[f"{pre}wo"], res[f], res_tag))
            return outs

        def dump_partial(res):
            for f in range(NSLOT):
                for tt in range(2):
                    row = 128 * (2 * f + tt)
                    nc.sync.dma_start(out.ap()[row:row + 128, 0:D], res[f][tt][:])

        if stop_stage == 1:
            dump_partial(r1)
            r2 = None
        else:
            r2 = axial_stage("r", r1, "r2", "s2")
        if debug and r2 is not None:
            nc.sync.dma_start(dbg["d_r2"].ap()[:], r2[0][0][:])
        if stop_stage == 2 and r2 is not None:
            dump_partial(r2)
        r3 = axial_stage("l", r2, "r3", "s3") if stop_stage >= 3 else None
        if debug and r3 is not None:
            nc.sync.dma_start(dbg["d_r3"].ap()[:], r3[0][0][:])

        # ================= stage 4: FFN + head =================
        if stop_stage == 3 and r3 is not None:
            dump_partial(r3)
        xT4, h1gs = [], []
        for f in range(NSLOT if stop_stage >= 4 else 0):
            aggs4, rs4 = ln_pair(r3[f][0], r3[f][1], f"s4{f}")
            xh = [ln_apply(r3[f][tt], aggs4[tt], rs4, tt, "s4")
                  for tt in range(2)]
            xT4.append(transpose_pair(xh, "s4"))
        for f in range(NSLOT if stop_stage >= 4 else 0):
            xT = xT4[f]
            h1g = []
            for qq in range(3):
                ph = pg(128, 512)
                for q2 in range(2):
                    q = 2 * qq + q2
                    for ch in range(2):
                        mm(ph[:, 256 * q2:256 * q2 + 256],
                           C["fw1"][ch][:, 128 * q:128 * q + 128],
                           xT[ch][:], start=(ch == 0), stop=(ch == 1))
                hg = dyn.tile([128, 512], BF16, tag=f"h1g{qq}", bufs=5)
                nc.scalar.activation(hg[:], ph[:], gelu_f)
                h1g.append(hg)
            if debug and f == 0:
                nc.sync.dma_start(dbg["d_h1g"].ap()[:], h1g[0][:, 0:256])
            h1gs.append(h1g)
        for f in range(NSLOT if stop_stage >= 4 else 0):
            h1g = h1gs[f]
            z = []
            for tt in range(2):
                pz = pg(128, D)
                for q in range(6):
                    mm(pz[:], h1g[q // 2][:, 256 * (q % 2) + 128 * tt:
                                          256 * (q % 2) + 128 * tt + 128],
                       C["fw2"][:, D * q:D * q + D],
                       start=(q == 0), stop=(q == 5))
                zt = dy3.tile([128, D], F32, tag="z_t")
                nc.vector.tensor_tensor(zt[:], pz[:], C["fb2"][:], OP.add)
                z_sb = st.tile([128, D], F32, tag=f"z_{f}_{tt}")
                nc.vector.tensor_tensor(z_sb[:], zt[:], r3[f][tt][:], OP.add)
                z.append(z_sb)
            if debug and f == 0:
                nc.sync.dma_start(dbg["d_z"].ap()[:], z[0][:])
            zT = transpose_pair(z, "hz", ones_row=False)
            for tt in range(2):
                po = pg(128, QL)
                for ch in range(2):
                    mm(po[:], zT[ch][0:96, 128 * tt:128 * tt + 128],
                       C["hw"][:, QL * ch:QL * ch + QL],
                       start=(ch == 0), stop=(ch == 1))
                ot = dy3.tile([128, QL], F32, tag="o_t")
                nc.vector.tensor_tensor(ot[:], po[:], C["hb"][:], OP.add)
                row = 128 * (2 * f + tt)
                nc.sync.dma_start(out.ap()[row:row + 128, :], ot[:])

    nc.compile()
    _CACHE[key] = nc
    return nc


# ---------------------------------------------------------------- entry

def kernel(**inputs):
    inputs = {k: np.asarray(v, np.float32) for k, v in inputs.items()}
    nc = build_program()
    const = _host_constants(inputs)
    in_maps = [_core_inputs(inputs, const, c) for c in range(NCORE)]
    res = bass_utils.run_bass_kernel_spmd(nc, in_maps, core_ids=list(range(NCORE)))
    out = np.zeros((B, T - 1, H, W, QL), np.float32)
    for f in range(30):
        b, t = _frame(f)
        core, j = f % 8, f // 8
        o = res.results[core]["out"].reshape(NSLOT, 2 * 128, QL)
        out[b, t - 1] = o[j].reshape(H, W, QL)
    return out

